# revision 25
# baseline (speedup 1.0000x reference)
"""Trainium2 Bass kernel for FlattenIntraCycleMoELayer (top-2 MoE + general path).

Strategy (v2):
  - Data-parallel over B (8 batteries per core).
  - gen_W is folded into each expert on host (gates sum to 1), so each
    battery is ONE fused matmul set: out = x @ (g1*A_e1 + g2*A_e2),
    A_e = gen_W + expert_W[e], bias folded via an appended ones-row.
  - Gating layer-1 is d_ff-sharded (fp32r matmul, 1 cyc/row); partial
    logits are exchanged core-to-core with remote_dma_broadcast (7
    single-dest sends into per-peer slots) instead of the ncfw
    AllReduce, then tree-summed locally.
  - Unpadded K=901 (7 full k-tiles + 5-row remainder), partition-major
    host layouts so every big DMA is contiguous per partition.
  - Output written bf16 (host casts to f32).
  - Main loop: kt-major matmuls per battery (dense PE stream), combine
    one battery ahead (t1 on DVE, t2 on ACT, add on DVE), evictions
    split DVE/ACT, PSUM double-buffered.

Host-side prep only reshapes/pads/casts/re-parametrizes weights
(elementwise adds of gen_W/gen_b into expert weights); all model math
runs on device.
"""

import numpy as np
import ml_dtypes


def _ensure_import_path():
    try:
        import concourse  # noqa: F401
    except ImportError:
        import sys
        for p in ("/opt/trn_rl_repo", "/root/.axon_site/_ro/trn_rl_repo"):
            if p not in sys.path:
                sys.path.insert(0, p)
        import concourse  # noqa: F401


_ensure_import_path()

import concourse.bass as bass  # noqa: E402
import concourse.tile as tile  # noqa: E402
from concourse import mybir  # noqa: E402
from concourse.bass import ds, ts  # noqa: E402
from concourse.alu_op_type import AluOpType  # noqa: E402
from concourse.masks import make_identity  # noqa: E402
from concourse.tile import add_dep_helper  # noqa: E402

BF16 = mybir.dt.bfloat16
F32 = mybir.dt.float32
F32R = mybir.dt.float32r
U32 = mybir.dt.uint32

# Problem shape constants (hardcoded per contest rules).
B, L, C, F = 64, 512, 3, 300
CF = C * F              # 900
K = CF + 1              # 901 contraction rows (data + ones row for bias)
KT = 8                  # k-tiles: 7 full + 1 remainder
KREM = K - 7 * 128      # 5 rows in the last k-tile
D = 512                 # d_model
E = 8                   # experts
NCORES = 8
BPC = B // NCORES       # 8 batteries per core
DLLM = 4096
GK = 4224               # padded gating contraction = 33*128
GKT = GK // 128         # 33
GA = 17                 # w1 k-tiles on ring A (sync)
GB = GKT - GA           # 16 k-tiles on ring B (scalar)
DFF = 2048
DFFC = DFF // NCORES    # 256 per-core d_ff chunk
EPS = 1e-9
MT = L // 128           # 4 m-tiles per battery


def build_program(nc):
    from contextlib import ExitStack

    xmain = nc.dram_tensor("xmain", [BPC, 128, 7, L], BF16, kind="ExternalInput")
    xrem = nc.dram_tensor("xrem", [BPC, KREM, L], BF16, kind="ExternalInput")
    amain = nc.dram_tensor("amain", [128, E, 7, D], BF16, kind="ExternalInput")
    arem = nc.dram_tensor("arem", [KREM, E, D], BF16, kind="ExternalInput")
    gintp = nc.dram_tensor("gintp", [128, GKT * B], F32R, kind="ExternalInput")
    w1a = nc.dram_tensor("w1a", [128, GA * DFFC], F32R, kind="ExternalInput")
    w1b = nc.dram_tensor("w1b", [128, GB * DFFC], F32R, kind="ExternalInput")
    w2p = nc.dram_tensor("w2p", [128, 2 * E], F32, kind="ExternalInput")
    seltd = nc.dram_tensor("selt", [B, BPC], F32, kind="ExternalInput")
    b2d = nc.dram_tensor("b2rep", [BPC, E], F32, kind="ExternalInput")
    sel2d = nc.dram_tensor("sel2", [2, 256], F32, kind="ExternalInput")
    outd = nc.dram_tensor("out", [BPC, 128, MT, D], BF16, kind="ExternalOutput")

    with tile.TileContext(nc) as tc, ExitStack() as ctx:
        singles = ctx.enter_context(tc.tile_pool(name="singles", bufs=1))
        gpool = ctx.enter_context(tc.tile_pool(name="gate", bufs=1))
        w1_ctx = ExitStack()
        w1pool = w1_ctx.enter_context(tc.tile_pool(name="w1s", bufs=1))
        gps_ctx = ExitStack()
        gps = gps_ctx.enter_context(tc.tile_pool(name="gpsum", bufs=1, space="PSUM"))

        # ------------- DMAs: ring A = nc.sync, ring B = nc.scalar -------
        # Ring A: gating inputs first, then even batteries' x.
        ginT_sb = gpool.tile([128, GKT, B], F32R)
        nc.sync.dma_start(out=ginT_sb.rearrange("p k b -> p (k b)"), in_=gintp.ap())
        w1a_sb = w1pool.tile([128, GA, DFFC], F32R)
        w1a_ap = w1a.ap().rearrange("p (k f) -> p k f", k=GA)
        nc.sync.dma_start(out=w1a_sb[:, 0:9, :], in_=w1a_ap[:, 0:9, :])
        nc.sync.dma_start(out=w1a_sb[:, 9:GA, :], in_=w1a_ap[:, 9:GA, :])
        selt_sb = gpool.tile([B, BPC], F32)
        nc.sync.dma_start(out=selt_sb, in_=seltd.ap())
        b2_sb = gpool.tile([BPC, E], F32)
        nc.sync.dma_start(out=b2_sb, in_=b2d.ap())
        w2_sb = gpool.tile([128, 2, E], F32)
        nc.sync.dma_start(out=w2_sb.rearrange("p j e -> p (j e)"), in_=w2p.ap())

        # Ring B: w1 second half, then the fused expert weights A.
        w1b_sb = w1pool.tile([128, GB, DFFC], F32R)
        w1b_ap = w1b.ap().rearrange("p (k f) -> p k f", k=GB)
        nc.scalar.dma_start(out=w1b_sb[:, 0:8, :], in_=w1b_ap[:, 0:8, :])
        nc.scalar.dma_start(out=w1b_sb[:, 8:GB, :], in_=w1b_ap[:, 8:GB, :])
        A_sb = singles.tile([128, E, KT, D], BF16)
        # zero the remainder k-tile first (only rows 0:KREM carry data; the
        # combine reads the full 128 partitions of it)
        nc.gpsimd.memset(A_sb[:, :, 7, :], 0.0)
        nc.scalar.dma_start(
            out=A_sb[:, :, 0:7, :],
            in_=amain.ap(),
        )
        nc.scalar.dma_start(
            out=A_sb[0:KREM, :, 7, :],
            in_=arem.ap(),
        )

        # x batteries: evens on ring A, odds on ring B (after the above).
        xmain_ap = xmain.ap()
        xrem_ap = xrem.ap()
        xb_tiles = []
        for b in range(BPC):
            eng = nc.sync if b % 2 == 0 else nc.scalar
            xb = singles.tile([128, KT, L], BF16, tag=f"xb{b}")
            eng.dma_start(
                out=xb[:, 0:7, :].rearrange("p k l -> p (k l)"),
                in_=xmain_ap[b].rearrange("p k l -> p (k l)"),
            )
            eng.dma_start(out=xb[0:KREM, 7, :], in_=xrem_ap[b])
            xb_tiles.append(xb)

        # ------------- constants / scratch for gating ------------------
        ident = singles.tile([128, 128], F32)
        make_identity(nc, ident)
        # SEL[:, 0:128] selects row 0 of a [2, N] rhs; SEL[:, 128:256] row 1.
        sel2 = singles.tile([2, 256], F32)
        nc.sync.dma_start(out=sel2, in_=sel2d.ap())

        # Cross-core logits exchange buffers.  Slot 0 = my partials,
        # slot j = partials from core (me XOR j).
        gather = gpool.tile([128, E, E], F32)
        gmset = nc.gpsimd.memset(gather[:, 0, :], 0.0)

        rsem = nc.alloc_semaphore("rsem")
        lsem = nc.alloc_semaphore("lsem")

        # ------------- gating layer 1 (fp32r, d_ff-sharded) ------------
        psum_h = gps.tile([B, DFFC], F32, bufs=1)
        order = [(w1a_sb, 0, 0, 9), (w1b_sb, GA, 0, 8),
                 (w1a_sb, 0, 9, GA), (w1b_sb, GA, 8, GB)]
        n_done = 0
        for (wt, base, lo, hi) in order:
            for k in range(lo, hi):
                kt_g = base + k
                nc.tensor.matmul(
                    out=psum_h, lhsT=ginT_sb[:, kt_g, :], rhs=wt[:, k, :],
                    start=(n_done == 0), stop=(n_done == GKT - 1),
                )
                n_done += 1

        # gelu (tanh approx):  h = 0.5*x*(1 + tanh(0.79788456*(x + 0.044715*x^3)))
        g_x = gpool.tile([B, DFFC], F32)
        nc.vector.tensor_copy(out=g_x, in_=psum_h)
        g_x2 = gpool.tile([B, DFFC], F32)
        nc.vector.tensor_tensor(out=g_x2, in0=g_x, in1=g_x, op=AluOpType.mult)
        g_p = gpool.tile([B, DFFC], F32)
        nc.vector.tensor_scalar(g_p, g_x2, 0.044715, 1.0,
                                AluOpType.mult, AluOpType.add)
        g_u = gpool.tile([B, DFFC], F32)
        nc.vector.tensor_tensor(out=g_u, in0=g_x, in1=g_p, op=AluOpType.mult)
        g_t = gpool.tile([B, DFFC], F32)
        nc.scalar.activation(out=g_t, in_=g_u,
                             func=mybir.ActivationFunctionType.Tanh,
                             scale=0.7978845608028654)
        g_q = gpool.tile([B, DFFC], F32)
        nc.vector.tensor_scalar(g_q, g_t, 1.0, 0.5,
                                AluOpType.add, AluOpType.mult)
        h_sb = gpool.tile([B, DFFC], F32)
        nc.vector.tensor_tensor(out=h_sb, in0=g_x, in1=g_q, op=AluOpType.mult)

        # transpose h -> hT [128, 2, B]
        hT_sb = gpool.tile([128, 2, B], F32)
        for j in range(2):
            pst = gps.tile([128, B], F32, bufs=2, tag="pst")
            nc.tensor.transpose(
                out=pst, in_=h_sb[:, j * 128:(j + 1) * 128], identity=ident[:B, :B]
            )
            nc.vector.tensor_copy(out=hT_sb[:, j, :], in_=pst)

        # layer 2 partial logits [B, E] -> gather slot 0
        psum_l = gps.tile([B, E], F32, bufs=2, tag="pst")
        for j in range(2):
            nc.tensor.matmul(out=psum_l, lhsT=hT_sb[:, j, :], rhs=w2_sb[:, j, :],
                             start=(j == 0), stop=(j == 1))
        nc.vector.tensor_copy(out=gather[0:B, 0, :], in_=psum_l)

        # exchange: 7 single-dest broadcasts (slot j <- core me XOR j), then
        # one trigger.  The wait for the 7 peers (2 sem incs each) is
        # injected post-scheduling onto su1 (see after the TileContext) so
        # Tile's single-core scheduling sim does not deadlock on a sem it
        # cannot see incremented.
        for j in range(1, NCORES):
            rd = [None] * 8
            rd[j] = (0, j)
            nc.gpsimd.remote_dma_broadcast(
                gather[:, j, :], gather[:, 0, :],
                remote_sem=rsem, local_sem=lsem, rdests=rd,
            )
        trig = nc.gpsimd.trigger_dma(count=None)
        add_dep_helper(trig.ins, gmset.ins, sync=True,
                       reason="src tile fully initialized before exchange")
        s4 = gpool.tile([128, 4, E], F32)
        su1 = nc.vector.tensor_tensor(
            out=s4.rearrange("p s e -> p (s e)"),
            in0=gather[:, 0:4, :].rearrange("p s e -> p (s e)"),
            in1=gather[:, 4:8, :].rearrange("p s e -> p (s e)"),
            op=AluOpType.add,
        )
        add_dep_helper(su1.ins, trig.ins, sync=False,
                       reason="sum scheduled after exchange trigger")
        su1_ins = su1.ins
        s2 = gpool.tile([128, 2, E], F32)
        nc.vector.tensor_tensor(
            out=s2.rearrange("p s e -> p (s e)"),
            in0=s4[:, 0:2, :].rearrange("p s e -> p (s e)"),
            in1=s4[:, 2:4, :].rearrange("p s e -> p (s e)"),
            op=AluOpType.add,
        )
        logits_all = gpool.tile([B, E], F32)
        nc.vector.tensor_tensor(out=logits_all, in0=s2[0:B, 0, :],
                                in1=s2[0:B, 1, :], op=AluOpType.add)

        # select my 8 batteries (one-hot matmul), add gate_b2
        psum_sel = gps.tile([BPC, E], F32, bufs=2, tag="pst")
        nc.tensor.matmul(out=psum_sel, lhsT=selt_sb, rhs=logits_all,
                         start=True, stop=True)
        logits_my = gpool.tile([BPC, E], F32)
        nc.vector.tensor_tensor(out=logits_my, in0=psum_sel, in1=b2_sb,
                                op=AluOpType.add)

        # top-2 gates: sorted values + indices, softmax renorm on top-2
        sorted8 = gpool.tile([BPC, E], F32)
        sidx = gpool.tile([BPC, E], U32)
        nc.vector.max(out=sorted8, in_=logits_my)
        nc.vector.max_index(out=sidx, in_max=sorted8, in_values=logits_my)
        negmax = gpool.tile([BPC, 1], F32)
        nc.vector.tensor_scalar_mul(negmax, sorted8[:, 0:1], -1.0)
        q = gpool.tile([BPC, E], F32)
        nc.scalar.activation(out=q, in_=sorted8,
                             func=mybir.ActivationFunctionType.Exp,
                             bias=negmax, scale=1.0)
        zsum = gpool.tile([BPC, 1], F32)
        nc.vector.reduce_sum(out=zsum, in_=q, axis=mybir.AxisListType.X)
        t12 = gpool.tile([BPC, 1], F32)
        nc.vector.tensor_tensor(out=t12, in0=q[:, 0:1], in1=q[:, 1:2],
                                op=AluOpType.add)
        den = gpool.tile([BPC, 1], F32)
        nc.vector.scalar_tensor_tensor(out=den, in0=zsum, scalar=EPS, in1=t12,
                                       op0=AluOpType.mult, op1=AluOpType.add)
        rden = gpool.tile([BPC, 1], F32)
        nc.vector.reciprocal(out=rden, in_=den)
        g12 = gpool.tile([BPC, 2], F32)
        nc.vector.tensor_scalar_mul(g12, q[:, 0:2], rden)

        # broadcast g1/g2 of each battery to all 128 partitions via PE:
        # transpose g12 -> [2, BPC], then ones-row matmuls.
        psum_tr = gps.tile([2, BPC], F32, bufs=2, tag="pst")
        nc.tensor.transpose(out=psum_tr, in_=g12, identity=ident[:BPC, :BPC])
        trs = gpool.tile([2, BPC], F32)
        nc.vector.tensor_copy(out=trs, in_=psum_tr)
        bcG = []
        for r in range(2):
            pbc = gps.tile([128, BPC], F32, bufs=2, tag="pbc")
            nc.tensor.matmul(out=pbc, lhsT=sel2[:, r * 128:(r + 1) * 128],
                             rhs=trs, start=True, stop=True)
            g_sb = gpool.tile([128, BPC], F32, tag=f"bcg{r}")
            nc.vector.tensor_copy(out=g_sb, in_=pbc)
            bcG.append(g_sb)

        gps_ctx.close()
        w1_ctx.close()

        # ------------- main fused phase --------------------------------
        mps = ctx.enter_context(tc.tile_pool(name="mpsum", bufs=2, space="PSUM"))
        wbpool = ctx.enter_context(tc.tile_pool(name="wbs", bufs=2))
        scpool = ctx.enter_context(tc.tile_pool(name="scratch", bufs=2))
        opool = ctx.enter_context(tc.tile_pool(name="outs", bufs=3))

        def _vload(eng, ap, name):
            reg = eng.alloc_register(name)
            eng.reg_load(reg, ap)
            val = eng.snap(reg, donate=True)
            return nc.s_assert_within(val, 0, E - 1, skip_runtime_assert=True)

        def combine(b):
            """wb = g1*A_e1 + g2*A_e2 for battery b (two kt-halves)."""
            rv1 = _vload(nc.vector, sidx[b:b + 1, 0:1], f"e1_{b}")
            rv2 = _vload(nc.scalar, sidx[b:b + 1, 1:2], f"e2_{b}")
            wb = wbpool.tile([128, KT, D], BF16)
            for h in range(2):
                kts = slice(h * 4, h * 4 + 4)
                t1 = scpool.tile([128, 4, D], BF16, tag="t1")
                nc.vector.tensor_scalar_mul(
                    t1.rearrange("p k d -> p (k d)"),
                    A_sb[:, ds(rv1, 1), kts, :].rearrange("p o k d -> p (o k d)"),
                    bcG[0][:, b:b + 1],
                )
                t2 = scpool.tile([128, 4, D], BF16, tag="t2")
                nc.scalar.activation(
                    out=t2.rearrange("p k d -> p (k d)"),
                    in_=A_sb[:, ds(rv2, 1), kts, :].rearrange("p o k d -> p (o k d)"),
                    func=mybir.ActivationFunctionType.Copy,
                    scale=bcG[1][:, b:b + 1],
                )
                nc.vector.tensor_tensor(
                    out=wb[:, kts, :].rearrange("p k d -> p (k d)"),
                    in0=t1.rearrange("p k d -> p (k d)"),
                    in1=t2.rearrange("p k d -> p (k d)"),
                    op=AluOpType.add,
                )
            return wb

        def battery(b, wb):
            xb = xb_tiles[b]
            pm = mps.tile([128, MT, D], F32, tag="mp")
            for kt in range(KT):
                np_ = KREM if kt == 7 else 128
                for m in range(MT):
                    nc.tensor.matmul(
                        out=pm[:, m, :],
                        lhsT=xb[0:np_, kt, ts(m, 128)],
                        rhs=wb[0:np_, kt, :],
                        start=(kt == 0), stop=(kt == KT - 1),
                    )
            osb = opool.tile([128, MT, D], BF16, tag="osb")
            for m in range(MT):
                if m < 2:
                    nc.vector.tensor_copy(out=osb[:, m, :], in_=pm[:, m, :])
                else:
                    nc.scalar.activation(out=osb[:, m, :], in_=pm[:, m, :],
                                         func=mybir.ActivationFunctionType.Copy)
            return nc.sync.dma_start(
                out=outd.ap()[b].rearrange("p m d -> p (m d)"),
                in_=osb.rearrange("p m d -> p (m d)"),
            )

        wbs = {0: combine(0), 1: combine(1)}
        for b in range(BPC):
            battery(b, wbs.pop(b))
            if b + 2 < BPC:
                wbs[b + 2] = combine(b + 2)

    # After the TileContext (which ends with drain + all-engine barrier):
    # reset the exchange semaphores so a second execution of this NEFF
    # starts from zero.  The lsem wait proves all 7 sends retired (16 local
    # increments each) before the clear.
    nc.gpsimd.wait_ge(lsem, 112)
    nc.gpsimd.drain()
    nc.all_engine_barrier()
    for s in (rsem, lsem):
        nc.gpsimd.dma_reset(range(s.num, s.num + 1))
        nc.gpsimd.sem_clear(range(s.num, s.num + 1))
    nc.all_engine_barrier()

    # Injected post-scheduling: su1 must wait for the 7 peers' partial
    # logits to land (2 rsem increments per peer).  Added here, after the
    # TileContext has scheduled, so the single-core scheduling sim never
    # blocks on a semaphore only remote cores increment.
    su1_ins.sync_info.on_wait.append(
        mybir.SyncWait(sync_type="semaphore", id=rsem.num,
                       wait_mode="sem-ge-imm", wait_value=14,
                       ant_name=rsem.name)
    )


def make_nc():
    from concourse import bacc
    nc = bacc.Bacc("TRN2", target_bir_lowering=False, debug=False,
                   num_devices=NCORES)
    build_program(nc)
    nc.finalize()
    return nc


def prep_inputs(cycle_curve_data, cycle_numbers, DKP_embeddings,
                gate_W1, gate_b1, gate_W2, gate_b2,
                expert_W, expert_b, gen_W, gen_b):
    """Host-side layout prep (reshape/pad/cast/weight-fold). Returns in_maps."""
    f32 = np.float32
    bf16 = ml_dtypes.bfloat16

    # fused expert weights A_e = gen_W + expert_W[e]; ones-row bias.
    A = np.empty((E, K, D), dtype=f32)
    A[:, :CF, :] = np.asarray(expert_W, dtype=f32) + np.asarray(gen_W, dtype=f32)
    A[:, CF, :] = np.asarray(expert_b, dtype=f32) + np.asarray(gen_b, dtype=f32)
    Abf = A.astype(bf16)
    amain = np.ascontiguousarray(
        Abf[:, :896, :].reshape(E, 7, 128, D).transpose(2, 0, 1, 3))
    arem = np.ascontiguousarray(Abf[:, 896:K, :].transpose(1, 0, 2))

    # x transposed with ones-row, partition-major.
    x = np.asarray(cycle_curve_data, dtype=f32).reshape(B, L, CF)
    xT = np.empty((B, K, L), dtype=bf16)
    xT[:, :CF, :] = x.transpose(0, 2, 1).astype(bf16)
    xT[:, CF, :] = np.asarray(1.0, dtype=bf16)
    xmain = np.ascontiguousarray(
        xT[:, :896, :].reshape(B, 7, 128, L).transpose(0, 2, 1, 3))
    xrem = np.ascontiguousarray(xT[:, 896:K, :])

    # gating input, partition-major [128, 33*64].
    g = np.zeros((GK, B), dtype=f32)
    g[:DLLM, :] = np.asarray(DKP_embeddings, dtype=f32).T
    g[DLLM, :] = np.asarray(cycle_numbers, dtype=f32)[:, 0]
    g[DLLM + 1, :] = 1.0
    gintp = np.ascontiguousarray(
        g.reshape(GKT, 128, B).transpose(1, 0, 2).reshape(128, GKT * B))

    W1p = np.zeros((GK, DFF), dtype=f32)
    W1p[:DLLM + 1, :] = np.asarray(gate_W1, dtype=f32)
    W1p[DLLM + 1, :] = np.asarray(gate_b1, dtype=f32)

    w2 = np.asarray(gate_W2, dtype=f32)
    b2rep = np.tile(np.asarray(gate_b2, dtype=f32).reshape(1, E), (BPC, 1))

    in_maps = []
    for c in range(NCORES):
        chunk = W1p[:, c * DFFC:(c + 1) * DFFC]
        w1pm = chunk.reshape(GKT, 128, DFFC).transpose(1, 0, 2)
        w1a = np.ascontiguousarray(w1pm[:, :GA, :].reshape(128, GA * DFFC))
        w1b = np.ascontiguousarray(w1pm[:, GA:, :].reshape(128, GB * DFFC))
        w2pm = np.ascontiguousarray(
            w2[c * DFFC:(c + 1) * DFFC, :].reshape(2, 128, E)
            .transpose(1, 0, 2).reshape(128, 2 * E))
        sel = np.zeros((B, BPC), dtype=f32)
        for i in range(BPC):
            sel[c * BPC + i, i] = 1.0
        sel2 = np.zeros((2, 256), dtype=f32)
        sel2[0, 0:128] = 1.0
        sel2[1, 128:256] = 1.0
        in_maps.append({
            "xmain": np.ascontiguousarray(xmain[c * BPC:(c + 1) * BPC]),
            "xrem": np.ascontiguousarray(xrem[c * BPC:(c + 1) * BPC]),
            "amain": amain,
            "arem": arem,
            "gintp": gintp,
            "w1a": w1a,
            "w1b": w1b,
            "w2p": w2pm,
            "selt": sel,
            "b2rep": b2rep,
            "sel2": sel2,
        })
    return in_maps


_CACHED = {}


def run(inputs, trace=False, tmpdir=None):
    """Run on the 8 NeuronCores; returns (full_output, BassKernelResults)."""
    from concourse import bass_utils
    in_maps = prep_inputs(**inputs)
    nc = _CACHED.get("nc")
    if nc is None:
        nc = make_nc()
        _CACHED["nc"] = nc
    res = bass_utils.run_bass_kernel_spmd(
        nc, in_maps, core_ids=list(range(NCORES)), trace=trace, tmpdir=tmpdir
    )
    outs = [np.asarray(r["out"]) for r in res.results]
    full = np.concatenate(outs, axis=0)          # [B, 128, MT, D] bf16
    full = full.transpose(0, 2, 1, 3).reshape(B, L, D).astype(np.float32)
    return full, res


def kernel(**inputs):
    full, _ = run(inputs, trace=False)
    return full


# revision 27
# speedup vs baseline: 36.7310x; 36.7310x over previous
"""Trainium2 Bass kernel for FlattenIntraCycleMoELayer (top-2 MoE + general path).

Strategy (v2):
  - Data-parallel over B (8 batteries per core).
  - gen_W is folded into each expert on host (gates sum to 1), so each
    battery is ONE fused matmul set: out = x @ (g1*A_e1 + g2*A_e2),
    A_e = gen_W + expert_W[e], bias folded via an appended ones-row.
  - Gating layer-1 is d_ff-sharded (fp32r matmul, 1 cyc/row); partial
    logits are exchanged core-to-core with remote_dma_broadcast (7
    single-dest sends into per-peer slots) instead of the ncfw
    AllReduce, then tree-summed locally.
  - Unpadded K=901 (7 full k-tiles + 5-row remainder), partition-major
    host layouts so every big DMA is contiguous per partition.
  - Output written bf16 (host casts to f32).
  - Main loop: kt-major matmuls per battery (dense PE stream), combine
    one battery ahead (t1 on DVE, t2 on ACT, add on DVE), evictions
    split DVE/ACT, PSUM double-buffered.

Host-side prep only reshapes/pads/casts/re-parametrizes weights
(elementwise adds of gen_W/gen_b into expert weights); all model math
runs on device.
"""

import numpy as np
import ml_dtypes


def _ensure_import_path():
    try:
        import concourse  # noqa: F401
    except ImportError:
        import sys
        for p in ("/opt/trn_rl_repo", "/root/.axon_site/_ro/trn_rl_repo"):
            if p not in sys.path:
                sys.path.insert(0, p)
        import concourse  # noqa: F401


_ensure_import_path()

import concourse.bass as bass  # noqa: E402
import concourse.tile as tile  # noqa: E402
from concourse import mybir  # noqa: E402
from concourse.bass import ds, ts  # noqa: E402
from concourse.alu_op_type import AluOpType  # noqa: E402
from concourse.masks import make_identity  # noqa: E402
from concourse.tile import add_dep_helper  # noqa: E402

BF16 = mybir.dt.bfloat16
F32 = mybir.dt.float32
F32R = mybir.dt.float32r
U32 = mybir.dt.uint32

# Problem shape constants (hardcoded per contest rules).
B, L, C, F = 64, 512, 3, 300
CF = C * F              # 900
K = CF + 1              # 901 contraction rows (data + ones row for bias)
KT = 8                  # k-tiles: 7 full + 1 remainder
KREM = K - 7 * 128      # 5 rows in the last k-tile
D = 512                 # d_model
E = 8                   # experts
NCORES = 8
BPC = B // NCORES       # 8 batteries per core
DLLM = 4096
GK = 4224               # padded gating contraction = 33*128
GKT = GK // 128         # 33
GA = 17                 # w1 k-tiles on ring A (sync)
GB = GKT - GA           # 16 k-tiles on ring B (scalar)
DFF = 2048
DFFC = DFF // NCORES    # 256 per-core d_ff chunk
EPS = 1e-9
MT = L // 128           # 4 m-tiles per battery


def build_program(nc):
    from contextlib import ExitStack

    xmain = nc.dram_tensor("xmain", [BPC, 128, 7, L], BF16, kind="ExternalInput")
    xrem = nc.dram_tensor("xrem", [BPC, KREM, L], BF16, kind="ExternalInput")
    amain = nc.dram_tensor("amain", [128, E, 7, D], BF16, kind="ExternalInput")
    arem = nc.dram_tensor("arem", [KREM, E, D], BF16, kind="ExternalInput")
    gintp = nc.dram_tensor("gintp", [128, GKT * B], F32R, kind="ExternalInput")
    w1a = nc.dram_tensor("w1a", [128, GA * DFFC], F32R, kind="ExternalInput")
    w1b = nc.dram_tensor("w1b", [128, GB * DFFC], F32R, kind="ExternalInput")
    w2p = nc.dram_tensor("w2p", [128, 2 * E], F32, kind="ExternalInput")
    seltd = nc.dram_tensor("selt", [B, BPC], F32, kind="ExternalInput")
    b2d = nc.dram_tensor("b2rep", [BPC, E], F32, kind="ExternalInput")
    sel2d = nc.dram_tensor("sel2", [2, 256], F32, kind="ExternalInput")
    outd = nc.dram_tensor("out", [BPC, 128, MT, D], BF16, kind="ExternalOutput")

    with tile.TileContext(nc) as tc, ExitStack() as ctx:
        singles = ctx.enter_context(tc.tile_pool(name="singles", bufs=1))
        gpool = ctx.enter_context(tc.tile_pool(name="gate", bufs=1))
        dpool = ctx.enter_context(tc.tile_pool(name="dram", bufs=1, space="DRAM"))

        # Fire-and-forget tiny AllReduce: registers this NEFF with the ncfw
        # collectives runtime so all 8 cores are gang-launched (otherwise
        # per-core launch skew reaches milliseconds and the logits exchange
        # stalls on it).  Runs async on the TOPSP firmware; result unused.
        cc_src = gpool.tile([1, 8], F32, tag="cc_src")
        nc.gpsimd.memset(cc_src, 0.0)
        cc_in = dpool.tile([1, 8], F32)
        nc.gpsimd.dma_start(out=cc_in, in_=cc_src)
        cc_out = dpool.tile([1, 8], F32, addr_space="Shared")
        nc.gpsimd.collective_compute(
            "AllReduce", AluOpType.add,
            replica_groups=[list(range(NCORES))],
            ins=[cc_in], outs=[cc_out],
        )
        w1_ctx = ExitStack()
        w1pool = w1_ctx.enter_context(tc.tile_pool(name="w1s", bufs=1))
        gps_ctx = ExitStack()
        gps = gps_ctx.enter_context(tc.tile_pool(name="gpsum", bufs=1, space="PSUM"))

        # ------------- DMAs: ring A = nc.sync, ring B = nc.scalar -------
        # Ring A: gating inputs first, then even batteries' x.
        ginT_sb = gpool.tile([128, GKT, B], F32R)
        nc.sync.dma_start(out=ginT_sb.rearrange("p k b -> p (k b)"), in_=gintp.ap())
        w1a_sb = w1pool.tile([128, GA, DFFC], F32R)
        w1a_ap = w1a.ap().rearrange("p (k f) -> p k f", k=GA)
        nc.sync.dma_start(out=w1a_sb[:, 0:9, :], in_=w1a_ap[:, 0:9, :])
        nc.sync.dma_start(out=w1a_sb[:, 9:GA, :], in_=w1a_ap[:, 9:GA, :])
        selt_sb = gpool.tile([B, BPC], F32)
        nc.sync.dma_start(out=selt_sb, in_=seltd.ap())
        b2_sb = gpool.tile([BPC, E], F32)
        nc.sync.dma_start(out=b2_sb, in_=b2d.ap())
        w2_sb = gpool.tile([128, 2, E], F32)
        nc.sync.dma_start(out=w2_sb.rearrange("p j e -> p (j e)"), in_=w2p.ap())

        # Ring B: w1 second half, then the fused expert weights A.
        w1b_sb = w1pool.tile([128, GB, DFFC], F32R)
        w1b_ap = w1b.ap().rearrange("p (k f) -> p k f", k=GB)
        nc.scalar.dma_start(out=w1b_sb[:, 0:8, :], in_=w1b_ap[:, 0:8, :])
        nc.scalar.dma_start(out=w1b_sb[:, 8:GB, :], in_=w1b_ap[:, 8:GB, :])
        A_sb = singles.tile([128, E, KT, D], BF16)
        # zero the remainder k-tile first (only rows 0:KREM carry data; the
        # combine reads the full 128 partitions of it)
        nc.gpsimd.memset(A_sb[:, :, 7, :], 0.0)
        nc.scalar.dma_start(
            out=A_sb[:, :, 0:7, :],
            in_=amain.ap(),
        )
        nc.scalar.dma_start(
            out=A_sb[0:KREM, :, 7, :],
            in_=arem.ap(),
        )

        # x batteries: evens on ring A, odds on ring B (after the above).
        xmain_ap = xmain.ap()
        xrem_ap = xrem.ap()
        xb_tiles = []
        for b in range(BPC):
            eng = nc.sync if b % 2 == 0 else nc.scalar
            xb = singles.tile([128, KT, L], BF16, tag=f"xb{b}")
            eng.dma_start(
                out=xb[:, 0:7, :].rearrange("p k l -> p (k l)"),
                in_=xmain_ap[b].rearrange("p k l -> p (k l)"),
            )
            eng.dma_start(out=xb[0:KREM, 7, :], in_=xrem_ap[b])
            xb_tiles.append(xb)

        # ------------- constants / scratch for gating ------------------
        ident = singles.tile([128, 128], F32)
        make_identity(nc, ident)
        # SEL[:, 0:128] selects row 0 of a [2, N] rhs; SEL[:, 128:256] row 1.
        sel2 = singles.tile([2, 256], F32)
        nc.sync.dma_start(out=sel2, in_=sel2d.ap())

        # Cross-core logits exchange buffers.  Slot 0 = my partials,
        # slot j = partials from core (me XOR j).
        gather = gpool.tile([128, E, E], F32)
        gmset = nc.gpsimd.memset(gather[:, 0, :], 0.0)

        rsem = nc.alloc_semaphore("rsem")
        lsem = nc.alloc_semaphore("lsem")

        # ------------- gating layer 1 (fp32r, d_ff-sharded) ------------
        psum_h = gps.tile([B, DFFC], F32, bufs=1)
        order = [(w1a_sb, 0, 0, 9), (w1b_sb, GA, 0, 8),
                 (w1a_sb, 0, 9, GA), (w1b_sb, GA, 8, GB)]
        n_done = 0
        for (wt, base, lo, hi) in order:
            for k in range(lo, hi):
                kt_g = base + k
                nc.tensor.matmul(
                    out=psum_h, lhsT=ginT_sb[:, kt_g, :], rhs=wt[:, k, :],
                    start=(n_done == 0), stop=(n_done == GKT - 1),
                )
                n_done += 1

        # gelu (tanh approx):  h = 0.5*x*(1 + tanh(0.79788456*(x + 0.044715*x^3)))
        g_x = gpool.tile([B, DFFC], F32)
        nc.vector.tensor_copy(out=g_x, in_=psum_h)
        g_x2 = gpool.tile([B, DFFC], F32)
        nc.vector.tensor_tensor(out=g_x2, in0=g_x, in1=g_x, op=AluOpType.mult)
        g_p = gpool.tile([B, DFFC], F32)
        nc.vector.tensor_scalar(g_p, g_x2, 0.044715, 1.0,
                                AluOpType.mult, AluOpType.add)
        g_u = gpool.tile([B, DFFC], F32)
        nc.vector.tensor_tensor(out=g_u, in0=g_x, in1=g_p, op=AluOpType.mult)
        g_t = gpool.tile([B, DFFC], F32)
        nc.scalar.activation(out=g_t, in_=g_u,
                             func=mybir.ActivationFunctionType.Tanh,
                             scale=0.7978845608028654)
        g_q = gpool.tile([B, DFFC], F32)
        nc.vector.tensor_scalar(g_q, g_t, 1.0, 0.5,
                                AluOpType.add, AluOpType.mult)
        h_sb = gpool.tile([B, DFFC], F32)
        nc.vector.tensor_tensor(out=h_sb, in0=g_x, in1=g_q, op=AluOpType.mult)

        # transpose h -> hT [128, 2, B]
        hT_sb = gpool.tile([128, 2, B], F32)
        for j in range(2):
            pst = gps.tile([128, B], F32, bufs=2, tag="pst")
            nc.tensor.transpose(
                out=pst, in_=h_sb[:, j * 128:(j + 1) * 128], identity=ident[:B, :B]
            )
            nc.vector.tensor_copy(out=hT_sb[:, j, :], in_=pst)

        # layer 2 partial logits [B, E] -> gather slot 0
        psum_l = gps.tile([B, E], F32, bufs=2, tag="pst")
        for j in range(2):
            nc.tensor.matmul(out=psum_l, lhsT=hT_sb[:, j, :], rhs=w2_sb[:, j, :],
                             start=(j == 0), stop=(j == 1))
        nc.vector.tensor_copy(out=gather[0:B, 0, :], in_=psum_l)

        # exchange: 7 single-dest broadcasts (slot j <- core me XOR j), then
        # one trigger.  The wait for the 7 peers (2 sem incs each) is
        # injected post-scheduling onto su1 (see after the TileContext) so
        # Tile's single-core scheduling sim does not deadlock on a sem it
        # cannot see incremented.
        for j in range(1, NCORES):
            rd = [None] * 8
            rd[j] = (0, j)
            nc.gpsimd.remote_dma_broadcast(
                gather[:, j, :], gather[:, 0, :],
                remote_sem=rsem, local_sem=lsem, rdests=rd,
            )
        trig = nc.gpsimd.trigger_dma(count=None)
        add_dep_helper(trig.ins, gmset.ins, sync=True,
                       reason="src tile fully initialized before exchange")
        s4 = gpool.tile([128, 4, E], F32)
        su1 = nc.vector.tensor_tensor(
            out=s4.rearrange("p s e -> p (s e)"),
            in0=gather[:, 0:4, :].rearrange("p s e -> p (s e)"),
            in1=gather[:, 4:8, :].rearrange("p s e -> p (s e)"),
            op=AluOpType.add,
        )
        add_dep_helper(su1.ins, trig.ins, sync=False,
                       reason="sum scheduled after exchange trigger")
        su1_ins = su1.ins
        s2 = gpool.tile([128, 2, E], F32)
        nc.vector.tensor_tensor(
            out=s2.rearrange("p s e -> p (s e)"),
            in0=s4[:, 0:2, :].rearrange("p s e -> p (s e)"),
            in1=s4[:, 2:4, :].rearrange("p s e -> p (s e)"),
            op=AluOpType.add,
        )
        logits_all = gpool.tile([B, E], F32)
        nc.vector.tensor_tensor(out=logits_all, in0=s2[0:B, 0, :],
                                in1=s2[0:B, 1, :], op=AluOpType.add)

        # select my 8 batteries (one-hot matmul), add gate_b2
        psum_sel = gps.tile([BPC, E], F32, bufs=2, tag="pst")
        nc.tensor.matmul(out=psum_sel, lhsT=selt_sb, rhs=logits_all,
                         start=True, stop=True)
        logits_my = gpool.tile([BPC, E], F32)
        nc.vector.tensor_tensor(out=logits_my, in0=psum_sel, in1=b2_sb,
                                op=AluOpType.add)

        # top-2 gates: sorted values + indices, softmax renorm on top-2
        sorted8 = gpool.tile([BPC, E], F32)
        sidx = gpool.tile([BPC, E], U32)
        nc.vector.max(out=sorted8, in_=logits_my)
        nc.vector.max_index(out=sidx, in_max=sorted8, in_values=logits_my)
        negmax = gpool.tile([BPC, 1], F32)
        nc.vector.tensor_scalar_mul(negmax, sorted8[:, 0:1], -1.0)
        q = gpool.tile([BPC, E], F32)
        nc.scalar.activation(out=q, in_=sorted8,
                             func=mybir.ActivationFunctionType.Exp,
                             bias=negmax, scale=1.0)
        zsum = gpool.tile([BPC, 1], F32)
        nc.vector.reduce_sum(out=zsum, in_=q, axis=mybir.AxisListType.X)
        t12 = gpool.tile([BPC, 1], F32)
        nc.vector.tensor_tensor(out=t12, in0=q[:, 0:1], in1=q[:, 1:2],
                                op=AluOpType.add)
        den = gpool.tile([BPC, 1], F32)
        nc.vector.scalar_tensor_tensor(out=den, in0=zsum, scalar=EPS, in1=t12,
                                       op0=AluOpType.mult, op1=AluOpType.add)
        rden = gpool.tile([BPC, 1], F32)
        nc.vector.reciprocal(out=rden, in_=den)
        g12 = gpool.tile([BPC, 2], F32)
        nc.vector.tensor_scalar_mul(g12, q[:, 0:2], rden)

        # broadcast g1/g2 of each battery to all 128 partitions via PE:
        # transpose g12 -> [2, BPC], then ones-row matmuls.
        psum_tr = gps.tile([2, BPC], F32, bufs=2, tag="pst")
        nc.tensor.transpose(out=psum_tr, in_=g12, identity=ident[:BPC, :BPC])
        trs = gpool.tile([2, BPC], F32)
        nc.vector.tensor_copy(out=trs, in_=psum_tr)
        bcG = []
        for r in range(2):
            pbc = gps.tile([128, BPC], F32, bufs=2, tag="pbc")
            nc.tensor.matmul(out=pbc, lhsT=sel2[:, r * 128:(r + 1) * 128],
                             rhs=trs, start=True, stop=True)
            g_sb = gpool.tile([128, BPC], F32, tag=f"bcg{r}")
            nc.vector.tensor_copy(out=g_sb, in_=pbc)
            bcG.append(g_sb)

        gps_ctx.close()
        w1_ctx.close()

        # ------------- main fused phase --------------------------------
        mps = ctx.enter_context(tc.tile_pool(name="mpsum", bufs=2, space="PSUM"))
        wbpool = ctx.enter_context(tc.tile_pool(name="wbs", bufs=2))
        scpool = ctx.enter_context(tc.tile_pool(name="scratch", bufs=2))
        opool = ctx.enter_context(tc.tile_pool(name="outs", bufs=3))

        def _vload(eng, ap, name):
            reg = eng.alloc_register(name)
            eng.reg_load(reg, ap)
            val = eng.snap(reg, donate=True)
            return nc.s_assert_within(val, 0, E - 1, skip_runtime_assert=True)

        def combine(b):
            """wb = g1*A_e1 + g2*A_e2 for battery b (two kt-halves)."""
            rv1 = _vload(nc.vector, sidx[b:b + 1, 0:1], f"e1_{b}")
            rv2 = _vload(nc.scalar, sidx[b:b + 1, 1:2], f"e2_{b}")
            wb = wbpool.tile([128, KT, D], BF16)
            for h in range(2):
                kts = slice(h * 4, h * 4 + 4)
                t1 = scpool.tile([128, 4, D], BF16, tag="t1")
                nc.vector.tensor_scalar_mul(
                    t1.rearrange("p k d -> p (k d)"),
                    A_sb[:, ds(rv1, 1), kts, :].rearrange("p o k d -> p (o k d)"),
                    bcG[0][:, b:b + 1],
                )
                t2 = scpool.tile([128, 4, D], BF16, tag="t2")
                nc.scalar.activation(
                    out=t2.rearrange("p k d -> p (k d)"),
                    in_=A_sb[:, ds(rv2, 1), kts, :].rearrange("p o k d -> p (o k d)"),
                    func=mybir.ActivationFunctionType.Copy,
                    scale=bcG[1][:, b:b + 1],
                )
                nc.vector.tensor_tensor(
                    out=wb[:, kts, :].rearrange("p k d -> p (k d)"),
                    in0=t1.rearrange("p k d -> p (k d)"),
                    in1=t2.rearrange("p k d -> p (k d)"),
                    op=AluOpType.add,
                )
            return wb

        def battery(b, wb):
            xb = xb_tiles[b]
            pm = mps.tile([128, MT, D], F32, tag="mp")
            for kt in range(KT):
                np_ = KREM if kt == 7 else 128
                for m in range(MT):
                    nc.tensor.matmul(
                        out=pm[:, m, :],
                        lhsT=xb[0:np_, kt, ts(m, 128)],
                        rhs=wb[0:np_, kt, :],
                        start=(kt == 0), stop=(kt == KT - 1),
                    )
            osb = opool.tile([128, MT, D], BF16, tag="osb")
            for m in range(MT):
                if m < 2:
                    nc.vector.tensor_copy(out=osb[:, m, :], in_=pm[:, m, :])
                else:
                    nc.scalar.activation(out=osb[:, m, :], in_=pm[:, m, :],
                                         func=mybir.ActivationFunctionType.Copy)
            return nc.sync.dma_start(
                out=outd.ap()[b].rearrange("p m d -> p (m d)"),
                in_=osb.rearrange("p m d -> p (m d)"),
            )

        wbs = {0: combine(0), 1: combine(1)}
        for b in range(BPC):
            battery(b, wbs.pop(b))
            if b + 2 < BPC:
                wbs[b + 2] = combine(b + 2)

    # After the TileContext (which ends with drain + all-engine barrier):
    # reset the exchange semaphores so a second execution of this NEFF
    # starts from zero.  The lsem wait proves all 7 sends retired (16 local
    # increments each) before the clear.
    nc.gpsimd.wait_ge(lsem, 112)
    nc.gpsimd.drain()
    nc.all_engine_barrier()
    for s in (rsem, lsem):
        nc.gpsimd.dma_reset(range(s.num, s.num + 1))
        nc.gpsimd.sem_clear(range(s.num, s.num + 1))
    nc.all_engine_barrier()

    # Injected post-scheduling: su1 must wait for the 7 peers' partial
    # logits to land (2 rsem increments per peer).  Added here, after the
    # TileContext has scheduled, so the single-core scheduling sim never
    # blocks on a semaphore only remote cores increment.
    su1_ins.sync_info.on_wait.append(
        mybir.SyncWait(sync_type="semaphore", id=rsem.num,
                       wait_mode="sem-ge-imm", wait_value=14,
                       ant_name=rsem.name)
    )


def make_nc():
    from concourse import bacc
    nc = bacc.Bacc("TRN2", target_bir_lowering=False, debug=False,
                   num_devices=NCORES)
    build_program(nc)
    nc.finalize()
    return nc


def prep_inputs(cycle_curve_data, cycle_numbers, DKP_embeddings,
                gate_W1, gate_b1, gate_W2, gate_b2,
                expert_W, expert_b, gen_W, gen_b):
    """Host-side layout prep (reshape/pad/cast/weight-fold). Returns in_maps."""
    f32 = np.float32
    bf16 = ml_dtypes.bfloat16

    # fused expert weights A_e = gen_W + expert_W[e]; ones-row bias.
    A = np.empty((E, K, D), dtype=f32)
    A[:, :CF, :] = np.asarray(expert_W, dtype=f32) + np.asarray(gen_W, dtype=f32)
    A[:, CF, :] = np.asarray(expert_b, dtype=f32) + np.asarray(gen_b, dtype=f32)
    Abf = A.astype(bf16)
    amain = np.ascontiguousarray(
        Abf[:, :896, :].reshape(E, 7, 128, D).transpose(2, 0, 1, 3))
    arem = np.ascontiguousarray(Abf[:, 896:K, :].transpose(1, 0, 2))

    # x transposed with ones-row, partition-major.
    x = np.asarray(cycle_curve_data, dtype=f32).reshape(B, L, CF)
    xT = np.empty((B, K, L), dtype=bf16)
    xT[:, :CF, :] = x.transpose(0, 2, 1).astype(bf16)
    xT[:, CF, :] = np.asarray(1.0, dtype=bf16)
    xmain = np.ascontiguousarray(
        xT[:, :896, :].reshape(B, 7, 128, L).transpose(0, 2, 1, 3))
    xrem = np.ascontiguousarray(xT[:, 896:K, :])

    # gating input, partition-major [128, 33*64].
    g = np.zeros((GK, B), dtype=f32)
    g[:DLLM, :] = np.asarray(DKP_embeddings, dtype=f32).T
    g[DLLM, :] = np.asarray(cycle_numbers, dtype=f32)[:, 0]
    g[DLLM + 1, :] = 1.0
    gintp = np.ascontiguousarray(
        g.reshape(GKT, 128, B).transpose(1, 0, 2).reshape(128, GKT * B))

    W1p = np.zeros((GK, DFF), dtype=f32)
    W1p[:DLLM + 1, :] = np.asarray(gate_W1, dtype=f32)
    W1p[DLLM + 1, :] = np.asarray(gate_b1, dtype=f32)

    w2 = np.asarray(gate_W2, dtype=f32)
    b2rep = np.tile(np.asarray(gate_b2, dtype=f32).reshape(1, E), (BPC, 1))

    in_maps = []
    for c in range(NCORES):
        chunk = W1p[:, c * DFFC:(c + 1) * DFFC]
        w1pm = chunk.reshape(GKT, 128, DFFC).transpose(1, 0, 2)
        w1a = np.ascontiguousarray(w1pm[:, :GA, :].reshape(128, GA * DFFC))
        w1b = np.ascontiguousarray(w1pm[:, GA:, :].reshape(128, GB * DFFC))
        w2pm = np.ascontiguousarray(
            w2[c * DFFC:(c + 1) * DFFC, :].reshape(2, 128, E)
            .transpose(1, 0, 2).reshape(128, 2 * E))
        sel = np.zeros((B, BPC), dtype=f32)
        for i in range(BPC):
            sel[c * BPC + i, i] = 1.0
        sel2 = np.zeros((2, 256), dtype=f32)
        sel2[0, 0:128] = 1.0
        sel2[1, 128:256] = 1.0
        in_maps.append({
            "xmain": np.ascontiguousarray(xmain[c * BPC:(c + 1) * BPC]),
            "xrem": np.ascontiguousarray(xrem[c * BPC:(c + 1) * BPC]),
            "amain": amain,
            "arem": arem,
            "gintp": gintp,
            "w1a": w1a,
            "w1b": w1b,
            "w2p": w2pm,
            "selt": sel,
            "b2rep": b2rep,
            "sel2": sel2,
        })
    return in_maps


_CACHED = {}


def run(inputs, trace=False, tmpdir=None):
    """Run on the 8 NeuronCores; returns (full_output, BassKernelResults)."""
    from concourse import bass_utils
    in_maps = prep_inputs(**inputs)
    nc = _CACHED.get("nc")
    if nc is None:
        nc = make_nc()
        _CACHED["nc"] = nc
    res = bass_utils.run_bass_kernel_spmd(
        nc, in_maps, core_ids=list(range(NCORES)), trace=trace, tmpdir=tmpdir
    )
    outs = [np.asarray(r["out"]) for r in res.results]
    full = np.concatenate(outs, axis=0)          # [B, 128, MT, D] bf16
    full = full.transpose(0, 2, 1, 3).reshape(B, L, D).astype(np.float32)
    return full, res


def kernel(**inputs):
    full, _ = run(inputs, trace=False)
    return full


# revision 32
# speedup vs baseline: 40.5381x; 1.1036x over previous
"""Trainium2 Bass kernel for FlattenIntraCycleMoELayer (top-2 MoE + general path).

Strategy (v2):
  - Data-parallel over B (8 batteries per core).
  - gen_W is folded into each expert on host (gates sum to 1), so each
    battery is ONE fused matmul set: out = x @ (g1*A_e1 + g2*A_e2),
    A_e = gen_W + expert_W[e], bias folded via an appended ones-row.
  - Gating layer-1 is d_ff-sharded (fp32r matmul, 1 cyc/row); partial
    logits are exchanged core-to-core with remote_dma_broadcast (7
    single-dest sends into per-peer slots) instead of the ncfw
    AllReduce, then tree-summed locally.
  - Unpadded K=901 (7 full k-tiles + 5-row remainder), partition-major
    host layouts so every big DMA is contiguous per partition.
  - Output written bf16 (host casts to f32).
  - Main loop: kt-major matmuls per battery (dense PE stream), combine
    one battery ahead (t1 on DVE, t2 on ACT, add on DVE), evictions
    split DVE/ACT, PSUM double-buffered.

Host-side prep only reshapes/pads/casts/re-parametrizes weights
(elementwise adds of gen_W/gen_b into expert weights); all model math
runs on device.
"""

import numpy as np
import ml_dtypes


def _ensure_import_path():
    try:
        import concourse  # noqa: F401
    except ImportError:
        import sys
        for p in ("/opt/trn_rl_repo", "/root/.axon_site/_ro/trn_rl_repo"):
            if p not in sys.path:
                sys.path.insert(0, p)
        import concourse  # noqa: F401


_ensure_import_path()

import concourse.bass as bass  # noqa: E402
import concourse.tile as tile  # noqa: E402
from concourse import mybir  # noqa: E402
from concourse.bass import ds, ts  # noqa: E402
from concourse.alu_op_type import AluOpType  # noqa: E402
from concourse.masks import make_identity  # noqa: E402
from concourse.tile import add_dep_helper  # noqa: E402

BF16 = mybir.dt.bfloat16
F32 = mybir.dt.float32
F32R = mybir.dt.float32r
U32 = mybir.dt.uint32

# Problem shape constants (hardcoded per contest rules).
B, L, C, F = 64, 512, 3, 300
CF = C * F              # 900
K = CF + 1              # 901 contraction rows (data + ones row for bias)
KT = 8                  # k-tiles: 7 full + 1 remainder
KREM = K - 7 * 128      # 5 rows in the last k-tile
D = 512                 # d_model
E = 8                   # experts
NCORES = 8
BPC = B // NCORES       # 8 batteries per core
DLLM = 4096
GK = 4224               # padded gating contraction = 33*128
GKT = GK // 128         # 33
GA = 17                 # w1 k-tiles on ring A (sync)
GB = GKT - GA           # 16 k-tiles on ring B (scalar)
DFF = 2048
DFFC = DFF // NCORES    # 256 per-core d_ff chunk
EPS = 1e-9
MT = L // 128           # 4 m-tiles per battery


def build_program(nc):
    from contextlib import ExitStack

    xmain = nc.dram_tensor("xmain", [BPC, 128, 7, L], BF16, kind="ExternalInput")
    xrem = nc.dram_tensor("xrem", [BPC, KREM, L], BF16, kind="ExternalInput")
    amain = nc.dram_tensor("amain", [128, E, 7, D], BF16, kind="ExternalInput")
    arem = nc.dram_tensor("arem", [KREM, E, D], BF16, kind="ExternalInput")
    gintp = nc.dram_tensor("gintp", [128, GKT * B], F32R, kind="ExternalInput")
    w1a = nc.dram_tensor("w1a", [128, GA * DFFC], F32R, kind="ExternalInput")
    w1b = nc.dram_tensor("w1b", [128, GB * DFFC], F32R, kind="ExternalInput")
    w2p = nc.dram_tensor("w2p", [128, 2 * E], F32, kind="ExternalInput")
    seltd = nc.dram_tensor("selt", [B, BPC], F32, kind="ExternalInput")
    b2d = nc.dram_tensor("b2rep", [BPC, E], F32, kind="ExternalInput")
    sel2d = nc.dram_tensor("sel2", [2, 256], F32, kind="ExternalInput")
    outd = nc.dram_tensor("out", [BPC, 128, MT, D], BF16, kind="ExternalOutput")

    with tile.TileContext(nc) as tc, ExitStack() as ctx:
        singles = ctx.enter_context(tc.tile_pool(name="singles", bufs=1))
        gpool = ctx.enter_context(tc.tile_pool(name="gate", bufs=1))
        dpool = ctx.enter_context(tc.tile_pool(name="dram", bufs=1, space="DRAM"))


        w1_ctx = ExitStack()
        w1pool = w1_ctx.enter_context(tc.tile_pool(name="w1s", bufs=1))
        gps_ctx = ExitStack()
        gps = gps_ctx.enter_context(tc.tile_pool(name="gpsum", bufs=1, space="PSUM"))

        # ------------- DMAs: ring A = nc.sync, ring B = nc.scalar -------
        # Ring A: gating inputs first, then even batteries' x.
        ginT_sb = gpool.tile([128, GKT, B], F32R)
        nc.sync.dma_start(out=ginT_sb.rearrange("p k b -> p (k b)"), in_=gintp.ap())
        w1a_sb = w1pool.tile([128, GA, DFFC], F32R)
        w1a_ap = w1a.ap().rearrange("p (k f) -> p k f", k=GA)
        nc.sync.dma_start(out=w1a_sb[:, 0:9, :], in_=w1a_ap[:, 0:9, :])
        nc.sync.dma_start(out=w1a_sb[:, 9:GA, :], in_=w1a_ap[:, 9:GA, :])
        selt_sb = gpool.tile([B, BPC], F32)
        nc.sync.dma_start(out=selt_sb, in_=seltd.ap())
        b2_sb = gpool.tile([BPC, E], F32)
        nc.sync.dma_start(out=b2_sb, in_=b2d.ap())
        w2_sb = gpool.tile([128, 2, E], F32)
        nc.sync.dma_start(out=w2_sb.rearrange("p j e -> p (j e)"), in_=w2p.ap())

        # Ring B: w1 second half, then the fused expert weights A.
        w1b_sb = w1pool.tile([128, GB, DFFC], F32R)
        w1b_ap = w1b.ap().rearrange("p (k f) -> p k f", k=GB)
        nc.scalar.dma_start(out=w1b_sb[:, 0:8, :], in_=w1b_ap[:, 0:8, :])
        nc.scalar.dma_start(out=w1b_sb[:, 8:GB, :], in_=w1b_ap[:, 8:GB, :])
        A_sb = singles.tile([128, E, KT, D], BF16)
        # zero the remainder k-tile first (only rows 0:KREM carry data; the
        # combine reads the full 128 partitions of it)
        nc.gpsimd.memset(A_sb[:, :, 7, :], 0.0)
        nc.scalar.dma_start(
            out=A_sb[:, :, 0:7, :],
            in_=amain.ap(),
        )
        nc.scalar.dma_start(
            out=A_sb[0:KREM, :, 7, :],
            in_=arem.ap(),
        )

        # x batteries: evens on ring A, odds on ring B (after the above).
        xmain_ap = xmain.ap()
        xrem_ap = xrem.ap()
        xb_tiles = []
        for b in range(BPC):
            eng = nc.sync if b % 2 == 0 else nc.scalar
            xb = singles.tile([128, KT, L], BF16, tag=f"xb{b}")
            eng.dma_start(
                out=xb[:, 0:7, :].rearrange("p k l -> p (k l)"),
                in_=xmain_ap[b].rearrange("p k l -> p (k l)"),
            )
            eng.dma_start(out=xb[0:KREM, 7, :], in_=xrem_ap[b])
            xb_tiles.append(xb)

        # ------------- constants / scratch for gating ------------------
        ident = singles.tile([128, 128], F32)
        make_identity(nc, ident)
        # SEL[:, 0:128] selects row 0 of a [2, N] rhs; SEL[:, 128:256] row 1.
        sel2 = singles.tile([2, 256], F32)
        nc.sync.dma_start(out=sel2, in_=sel2d.ap())

        # Cross-core logits exchange buffers.  Slot 0 = my partials,
        # slot j = partials from core (me XOR j).
        gather = gpool.tile([128, E, E], F32)
        gmset = nc.gpsimd.memset(gather[:, 0, :], 0.0)

        rsem = nc.alloc_semaphore("rsem")
        lsem = nc.alloc_semaphore("lsem")

        # ------------- gating layer 1 (fp32r, d_ff-sharded) ------------
        psum_h = gps.tile([B, DFFC], F32, bufs=1)
        order = [(w1a_sb, 0, 0, 9), (w1b_sb, GA, 0, 8),
                 (w1a_sb, 0, 9, GA), (w1b_sb, GA, 8, GB)]
        n_done = 0
        for (wt, base, lo, hi) in order:
            for k in range(lo, hi):
                kt_g = base + k
                nc.tensor.matmul(
                    out=psum_h, lhsT=ginT_sb[:, kt_g, :], rhs=wt[:, k, :],
                    start=(n_done == 0), stop=(n_done == GKT - 1),
                )
                n_done += 1

        # gelu (tanh approx):  h = 0.5*x*(1 + tanh(0.79788456*(x + 0.044715*x^3)))
        g_x = gpool.tile([B, DFFC], F32)
        nc.vector.tensor_copy(out=g_x, in_=psum_h)
        g_x2 = gpool.tile([B, DFFC], F32)
        nc.vector.tensor_tensor(out=g_x2, in0=g_x, in1=g_x, op=AluOpType.mult)
        g_p = gpool.tile([B, DFFC], F32)
        nc.vector.tensor_scalar(g_p, g_x2, 0.044715, 1.0,
                                AluOpType.mult, AluOpType.add)
        g_u = gpool.tile([B, DFFC], F32)
        nc.vector.tensor_tensor(out=g_u, in0=g_x, in1=g_p, op=AluOpType.mult)
        g_t = gpool.tile([B, DFFC], F32)
        nc.scalar.activation(out=g_t, in_=g_u,
                             func=mybir.ActivationFunctionType.Tanh,
                             scale=0.7978845608028654)
        g_q = gpool.tile([B, DFFC], F32)
        nc.vector.tensor_scalar(g_q, g_t, 1.0, 0.5,
                                AluOpType.add, AluOpType.mult)
        h_sb = gpool.tile([B, DFFC], F32)
        nc.vector.tensor_tensor(out=h_sb, in0=g_x, in1=g_q, op=AluOpType.mult)

        # transpose h -> hT [128, 2, B]
        hT_sb = gpool.tile([128, 2, B], F32)
        for j in range(2):
            pst = gps.tile([128, B], F32, bufs=2, tag="pst")
            nc.tensor.transpose(
                out=pst, in_=h_sb[:, j * 128:(j + 1) * 128], identity=ident[:B, :B]
            )
            nc.vector.tensor_copy(out=hT_sb[:, j, :], in_=pst)

        # layer 2 partial logits [B, E] -> gather slot 0
        psum_l = gps.tile([B, E], F32, bufs=2, tag="pst")
        for j in range(2):
            nc.tensor.matmul(out=psum_l, lhsT=hT_sb[:, j, :], rhs=w2_sb[:, j, :],
                             start=(j == 0), stop=(j == 1))
        nc.vector.tensor_copy(out=gather[0:B, 0, :], in_=psum_l)

        # exchange: 7 single-dest broadcasts (slot j <- core me XOR j), then
        # one trigger.  The wait for the 7 peers (2 sem incs each) is
        # injected post-scheduling onto su1 (see after the TileContext) so
        # Tile's single-core scheduling sim does not deadlock on a sem it
        # cannot see incremented.
        for j in range(1, NCORES):
            rd = [None] * 8
            rd[j] = (0, j)
            nc.gpsimd.remote_dma_broadcast(
                gather[:, j, :], gather[:, 0, :],
                remote_sem=rsem, local_sem=lsem, rdests=rd,
            )
        trig = nc.gpsimd.trigger_dma(count=None)
        add_dep_helper(trig.ins, gmset.ins, sync=True,
                       reason="src tile fully initialized before exchange")

        # Fire-and-forget tiny AllReduce AFTER the exchange trigger in the
        # gpsimd stream (its ncfw barrier blocks gpsimd for ~40us).  Its only
        # job: registering this NEFF with the collectives runtime so the 8
        # cores are gang-launched — without it, per-core launch skew reaches
        # milliseconds and the logits exchange stalls on it.  Result unused;
        # the barrier+reduce run on TOPSP firmware concurrent with the main
        # fused phase.
        cc_src = gpool.tile([1, 8], F32, tag="cc_src")
        nc.gpsimd.memset(cc_src, 0.0)
        cc_in = dpool.tile([1, 8], F32)
        cc_dma = nc.gpsimd.dma_start(out=cc_in, in_=cc_src)
        add_dep_helper(cc_dma.ins, trig.ins, sync=False,
                       reason="keep collective path after exchange trigger")
        cc_out = dpool.tile([1, 8], F32, addr_space="Shared")
        nc.gpsimd.collective_compute(
            "AllReduce", AluOpType.add,
            replica_groups=[list(range(NCORES))],
            ins=[cc_in], outs=[cc_out],
        )
        s4 = gpool.tile([128, 4, E], F32)
        su1 = nc.vector.tensor_tensor(
            out=s4.rearrange("p s e -> p (s e)"),
            in0=gather[:, 0:4, :].rearrange("p s e -> p (s e)"),
            in1=gather[:, 4:8, :].rearrange("p s e -> p (s e)"),
            op=AluOpType.add,
        )
        add_dep_helper(su1.ins, trig.ins, sync=False,
                       reason="sum scheduled after exchange trigger")
        su1_ins = su1.ins

        # PE warm-up: ~4us of junk matmuls starting right after the
        # exchange wait passes, so the HAM clock-gate is at K=8/8 when the
        # fused matmuls begin (the PE idles while waiting for peers and
        # gets re-throttled to 1.2 GHz otherwise).  Results overwrite the
        # dead gating PSUM tile.
        for j in range(16):
            jmm = nc.tensor.matmul(
                out=psum_h, lhsT=ginT_sb[:, j, :], rhs=w1a_sb[:, j, :],
                start=True, stop=True,
            )
            if j == 0:
                add_dep_helper(jmm.ins, su1.ins, sync=True,
                               reason="warm-up matmuls start at exchange completion")
        s2 = gpool.tile([128, 2, E], F32)
        nc.vector.tensor_tensor(
            out=s2.rearrange("p s e -> p (s e)"),
            in0=s4[:, 0:2, :].rearrange("p s e -> p (s e)"),
            in1=s4[:, 2:4, :].rearrange("p s e -> p (s e)"),
            op=AluOpType.add,
        )
        logits_all = gpool.tile([B, E], F32)
        nc.vector.tensor_tensor(out=logits_all, in0=s2[0:B, 0, :],
                                in1=s2[0:B, 1, :], op=AluOpType.add)

        # select my 8 batteries (one-hot matmul), add gate_b2
        psum_sel = gps.tile([BPC, E], F32, bufs=2, tag="pst")
        nc.tensor.matmul(out=psum_sel, lhsT=selt_sb, rhs=logits_all,
                         start=True, stop=True)
        logits_my = gpool.tile([BPC, E], F32)
        nc.vector.tensor_tensor(out=logits_my, in0=psum_sel, in1=b2_sb,
                                op=AluOpType.add)

        # top-2 gates: sorted values + indices, softmax renorm on top-2
        sorted8 = gpool.tile([BPC, E], F32)
        sidx = gpool.tile([BPC, E], U32)
        nc.vector.max(out=sorted8, in_=logits_my)
        nc.vector.max_index(out=sidx, in_max=sorted8, in_values=logits_my)
        negmax = gpool.tile([BPC, 1], F32)
        nc.vector.tensor_scalar_mul(negmax, sorted8[:, 0:1], -1.0)
        q = gpool.tile([BPC, E], F32)
        nc.scalar.activation(out=q, in_=sorted8,
                             func=mybir.ActivationFunctionType.Exp,
                             bias=negmax, scale=1.0)
        zsum = gpool.tile([BPC, 1], F32)
        nc.vector.reduce_sum(out=zsum, in_=q, axis=mybir.AxisListType.X)
        t12 = gpool.tile([BPC, 1], F32)
        nc.vector.tensor_tensor(out=t12, in0=q[:, 0:1], in1=q[:, 1:2],
                                op=AluOpType.add)
        den = gpool.tile([BPC, 1], F32)
        nc.vector.scalar_tensor_tensor(out=den, in0=zsum, scalar=EPS, in1=t12,
                                       op0=AluOpType.mult, op1=AluOpType.add)
        rden = gpool.tile([BPC, 1], F32)
        nc.vector.reciprocal(out=rden, in_=den)
        g12 = gpool.tile([BPC, 2], F32)
        nc.vector.tensor_scalar_mul(g12, q[:, 0:2], rden)

        # broadcast g1/g2 of each battery to all 128 partitions via PE:
        # transpose g12 -> [2, BPC], then ones-row matmuls.
        psum_tr = gps.tile([2, BPC], F32, bufs=2, tag="pst")
        nc.tensor.transpose(out=psum_tr, in_=g12, identity=ident[:BPC, :BPC])
        trs = gpool.tile([2, BPC], F32)
        nc.vector.tensor_copy(out=trs, in_=psum_tr)
        bcG = []
        for r in range(2):
            pbc = gps.tile([128, BPC], F32, bufs=2, tag="pbc")
            nc.tensor.matmul(out=pbc, lhsT=sel2[:, r * 128:(r + 1) * 128],
                             rhs=trs, start=True, stop=True)
            g_sb = gpool.tile([128, BPC], F32, tag=f"bcg{r}")
            nc.vector.tensor_copy(out=g_sb, in_=pbc)
            bcG.append(g_sb)

        gps_ctx.close()
        w1_ctx.close()

        # ------------- main fused phase --------------------------------
        mps = ctx.enter_context(tc.tile_pool(name="mpsum", bufs=2, space="PSUM"))
        wbpool = ctx.enter_context(tc.tile_pool(name="wbs", bufs=2))
        scpool = ctx.enter_context(tc.tile_pool(name="scratch", bufs=2))
        opool = ctx.enter_context(tc.tile_pool(name="outs", bufs=3))

        def _vload(eng, ap, name):
            reg = eng.alloc_register(name)
            eng.reg_load(reg, ap)
            val = eng.snap(reg, donate=True)
            return nc.s_assert_within(val, 0, E - 1, skip_runtime_assert=True)

        def combine(b):
            """wb = g1*A_e1 + g2*A_e2 for battery b (two kt-halves, DVE)."""
            rv1 = _vload(nc.vector, sidx[b:b + 1, 0:1], f"e1_{b}")
            rv2 = _vload(nc.vector, sidx[b:b + 1, 1:2], f"e2_{b}")
            wb = wbpool.tile([128, KT, D], BF16)
            for h in range(2):
                kts = slice(h * 4, h * 4 + 4)
                t2 = scpool.tile([128, 4, D], BF16, tag="t2")
                nc.vector.tensor_scalar_mul(
                    t2.rearrange("p k d -> p (k d)"),
                    A_sb[:, ds(rv2, 1), kts, :].rearrange("p o k d -> p (o k d)"),
                    bcG[1][:, b:b + 1],
                )
                nc.vector.scalar_tensor_tensor(
                    out=wb[:, kts, :].rearrange("p k d -> p (k d)"),
                    in0=A_sb[:, ds(rv1, 1), kts, :].rearrange("p o k d -> p (o k d)"),
                    scalar=bcG[0][:, b:b + 1],
                    in1=t2.rearrange("p k d -> p (k d)"),
                    op0=AluOpType.mult, op1=AluOpType.add,
                )
            return wb

        def battery(b, wb):
            xb = xb_tiles[b]
            pm = mps.tile([128, MT, D], F32, tag="mp")
            for kt in range(KT):
                np_ = KREM if kt == 7 else 128
                for m in range(MT):
                    nc.tensor.matmul(
                        out=pm[:, m, :],
                        lhsT=xb[0:np_, kt, ts(m, 128)],
                        rhs=wb[0:np_, kt, :],
                        start=(kt == 0), stop=(kt == KT - 1),
                    )
            osb = opool.tile([128, MT, D], BF16, tag="osb")
            for m in range(MT):
                nc.scalar.activation(out=osb[:, m, :], in_=pm[:, m, :],
                                     func=mybir.ActivationFunctionType.Copy)
            return nc.sync.dma_start(
                out=outd.ap()[b].rearrange("p m d -> p (m d)"),
                in_=osb.rearrange("p m d -> p (m d)"),
            )

        wbs = {0: combine(0), 1: combine(1)}
        for b in range(BPC):
            battery(b, wbs.pop(b))
            if b + 2 < BPC:
                wbs[b + 2] = combine(b + 2)

    # After the TileContext (which ends with drain + all-engine barrier):
    # reset the exchange semaphores so a second execution of this NEFF
    # starts from zero.  The lsem wait proves all 7 sends retired (16 local
    # increments each) before the clear.
    nc.gpsimd.wait_ge(lsem, 112)
    nc.gpsimd.drain()
    nc.all_engine_barrier()
    for s in (rsem, lsem):
        nc.gpsimd.dma_reset(range(s.num, s.num + 1))
        nc.gpsimd.sem_clear(range(s.num, s.num + 1))
    nc.all_engine_barrier()

    # Injected post-scheduling: su1 must wait for the 7 peers' partial
    # logits to land (2 rsem increments per peer).  Added here, after the
    # TileContext has scheduled, so the single-core scheduling sim never
    # blocks on a semaphore only remote cores increment.
    su1_ins.sync_info.on_wait.append(
        mybir.SyncWait(sync_type="semaphore", id=rsem.num,
                       wait_mode="sem-ge-imm", wait_value=14,
                       ant_name=rsem.name)
    )


def make_nc():
    from concourse import bacc
    nc = bacc.Bacc("TRN2", target_bir_lowering=False, debug=False,
                   num_devices=NCORES)
    build_program(nc)
    nc.finalize()
    return nc


def prep_inputs(cycle_curve_data, cycle_numbers, DKP_embeddings,
                gate_W1, gate_b1, gate_W2, gate_b2,
                expert_W, expert_b, gen_W, gen_b):
    """Host-side layout prep (reshape/pad/cast/weight-fold). Returns in_maps."""
    f32 = np.float32
    bf16 = ml_dtypes.bfloat16

    # fused expert weights A_e = gen_W + expert_W[e]; ones-row bias.
    A = np.empty((E, K, D), dtype=f32)
    A[:, :CF, :] = np.asarray(expert_W, dtype=f32) + np.asarray(gen_W, dtype=f32)
    A[:, CF, :] = np.asarray(expert_b, dtype=f32) + np.asarray(gen_b, dtype=f32)
    Abf = A.astype(bf16)
    amain = np.ascontiguousarray(
        Abf[:, :896, :].reshape(E, 7, 128, D).transpose(2, 0, 1, 3))
    arem = np.ascontiguousarray(Abf[:, 896:K, :].transpose(1, 0, 2))

    # x transposed with ones-row, partition-major.
    x = np.asarray(cycle_curve_data, dtype=f32).reshape(B, L, CF)
    xT = np.empty((B, K, L), dtype=bf16)
    xT[:, :CF, :] = x.transpose(0, 2, 1).astype(bf16)
    xT[:, CF, :] = np.asarray(1.0, dtype=bf16)
    xmain = np.ascontiguousarray(
        xT[:, :896, :].reshape(B, 7, 128, L).transpose(0, 2, 1, 3))
    xrem = np.ascontiguousarray(xT[:, 896:K, :])

    # gating input, partition-major [128, 33*64].
    g = np.zeros((GK, B), dtype=f32)
    g[:DLLM, :] = np.asarray(DKP_embeddings, dtype=f32).T
    g[DLLM, :] = np.asarray(cycle_numbers, dtype=f32)[:, 0]
    g[DLLM + 1, :] = 1.0
    gintp = np.ascontiguousarray(
        g.reshape(GKT, 128, B).transpose(1, 0, 2).reshape(128, GKT * B))

    W1p = np.zeros((GK, DFF), dtype=f32)
    W1p[:DLLM + 1, :] = np.asarray(gate_W1, dtype=f32)
    W1p[DLLM + 1, :] = np.asarray(gate_b1, dtype=f32)

    w2 = np.asarray(gate_W2, dtype=f32)
    b2rep = np.tile(np.asarray(gate_b2, dtype=f32).reshape(1, E), (BPC, 1))

    in_maps = []
    for c in range(NCORES):
        chunk = W1p[:, c * DFFC:(c + 1) * DFFC]
        w1pm = chunk.reshape(GKT, 128, DFFC).transpose(1, 0, 2)
        w1a = np.ascontiguousarray(w1pm[:, :GA, :].reshape(128, GA * DFFC))
        w1b = np.ascontiguousarray(w1pm[:, GA:, :].reshape(128, GB * DFFC))
        w2pm = np.ascontiguousarray(
            w2[c * DFFC:(c + 1) * DFFC, :].reshape(2, 128, E)
            .transpose(1, 0, 2).reshape(128, 2 * E))
        sel = np.zeros((B, BPC), dtype=f32)
        for i in range(BPC):
            sel[c * BPC + i, i] = 1.0
        sel2 = np.zeros((2, 256), dtype=f32)
        sel2[0, 0:128] = 1.0
        sel2[1, 128:256] = 1.0
        in_maps.append({
            "xmain": np.ascontiguousarray(xmain[c * BPC:(c + 1) * BPC]),
            "xrem": np.ascontiguousarray(xrem[c * BPC:(c + 1) * BPC]),
            "amain": amain,
            "arem": arem,
            "gintp": gintp,
            "w1a": w1a,
            "w1b": w1b,
            "w2p": w2pm,
            "selt": sel,
            "b2rep": b2rep,
            "sel2": sel2,
        })
    return in_maps


_CACHED = {}


def run(inputs, trace=False, tmpdir=None):
    """Run on the 8 NeuronCores; returns (full_output, BassKernelResults)."""
    from concourse import bass_utils
    in_maps = prep_inputs(**inputs)
    nc = _CACHED.get("nc")
    if nc is None:
        nc = make_nc()
        _CACHED["nc"] = nc
    res = bass_utils.run_bass_kernel_spmd(
        nc, in_maps, core_ids=list(range(NCORES)), trace=trace, tmpdir=tmpdir
    )
    outs = [np.asarray(r["out"]) for r in res.results]
    full = np.concatenate(outs, axis=0)          # [B, 128, MT, D] bf16
    full = full.transpose(0, 2, 1, 3).reshape(B, L, D).astype(np.float32)
    return full, res


def kernel(**inputs):
    full, _ = run(inputs, trace=False)
    return full


# revision 39
# speedup vs baseline: 41.0668x; 1.0130x over previous
"""Trainium2 Bass kernel for FlattenIntraCycleMoELayer (top-2 MoE + general path).

Strategy (v2):
  - Data-parallel over B (8 batteries per core).
  - gen_W is folded into each expert on host (gates sum to 1), so each
    battery is ONE fused matmul set: out = x @ (g1*A_e1 + g2*A_e2),
    A_e = gen_W + expert_W[e], bias folded via an appended ones-row.
  - Gating layer-1 is d_ff-sharded (fp32r matmul, 1 cyc/row); partial
    logits are exchanged core-to-core with remote_dma_broadcast (7
    single-dest sends into per-peer slots) instead of the ncfw
    AllReduce, then tree-summed locally.
  - Unpadded K=901 (7 full k-tiles + 5-row remainder), partition-major
    host layouts so every big DMA is contiguous per partition.
  - Output written bf16 (host casts to f32).
  - Main loop: kt-major matmuls per battery (dense PE stream), combine
    one battery ahead (t1 on DVE, t2 on ACT, add on DVE), evictions
    split DVE/ACT, PSUM double-buffered.

Host-side prep only reshapes/pads/casts/re-parametrizes weights
(elementwise adds of gen_W/gen_b into expert weights); all model math
runs on device.
"""

import numpy as np
import ml_dtypes


def _ensure_import_path():
    try:
        import concourse  # noqa: F401
    except ImportError:
        import sys
        for p in ("/opt/trn_rl_repo", "/root/.axon_site/_ro/trn_rl_repo"):
            if p not in sys.path:
                sys.path.insert(0, p)
        import concourse  # noqa: F401


_ensure_import_path()

import concourse.bass as bass  # noqa: E402
import concourse.tile as tile  # noqa: E402
from concourse import mybir  # noqa: E402
from concourse.bass import ds, ts  # noqa: E402
from concourse.alu_op_type import AluOpType  # noqa: E402
from concourse.masks import make_identity  # noqa: E402
from concourse.tile import add_dep_helper  # noqa: E402

BF16 = mybir.dt.bfloat16
F32 = mybir.dt.float32
F32R = mybir.dt.float32r
U32 = mybir.dt.uint32

# Problem shape constants (hardcoded per contest rules).
B, L, C, F = 64, 512, 3, 300
CF = C * F              # 900
K = CF + 1              # 901 contraction rows (data + ones row for bias)
KT = 8                  # k-tiles: 7 full + 1 remainder
KREM = K - 7 * 128      # 5 rows in the last k-tile
D = 512                 # d_model
E = 8                   # experts
NCORES = 8
BPC = B // NCORES       # 8 batteries per core
DLLM = 4096
GK = 4224               # padded gating contraction = 33*128
GKT = GK // 128         # 33
GA = 17                 # w1 k-tiles on ring A (sync)
GB = GKT - GA           # 16 k-tiles on ring B (scalar)
DFF = 2048
DFFC = DFF // NCORES    # 256 per-core d_ff chunk
EPS = 1e-9
MT = L // 128           # 4 m-tiles per battery


def build_program(nc):
    from contextlib import ExitStack

    xmain = nc.dram_tensor("xmain", [BPC, 128, 7, L], BF16, kind="ExternalInput")
    xrem = nc.dram_tensor("xrem", [BPC, KREM, L], BF16, kind="ExternalInput")
    amain = nc.dram_tensor("amain", [128, E, 7, D], BF16, kind="ExternalInput")
    arem = nc.dram_tensor("arem", [KREM, E, D], BF16, kind="ExternalInput")
    gintp = nc.dram_tensor("gintp", [128, GKT * B], F32R, kind="ExternalInput")
    w1a = nc.dram_tensor("w1a", [128, GA * DFFC], F32R, kind="ExternalInput")
    w1b = nc.dram_tensor("w1b", [128, GB * DFFC], F32R, kind="ExternalInput")
    w2p = nc.dram_tensor("w2p", [128, 2 * E], F32, kind="ExternalInput")
    seltd = nc.dram_tensor("selt", [B, BPC], F32, kind="ExternalInput")
    b2d = nc.dram_tensor("b2rep", [BPC, E], F32, kind="ExternalInput")
    sel2d = nc.dram_tensor("sel2", [2, 256], F32, kind="ExternalInput")
    outd = nc.dram_tensor("out", [BPC, 128, MT, D], BF16, kind="ExternalOutput")

    with tile.TileContext(nc) as tc, ExitStack() as ctx:
        singles = ctx.enter_context(tc.tile_pool(name="singles", bufs=1))
        gpool = ctx.enter_context(tc.tile_pool(name="gate", bufs=1))
        dpool = ctx.enter_context(tc.tile_pool(name="dram", bufs=1, space="DRAM"))


        w1_ctx = ExitStack()
        w1pool = w1_ctx.enter_context(tc.tile_pool(name="w1s", bufs=1))
        gps_ctx = ExitStack()
        gps = gps_ctx.enter_context(tc.tile_pool(name="gpsum", bufs=1, space="PSUM"))

        # ------------- DMAs: ring A = nc.sync, ring B = nc.scalar -------
        # Ring A: gating inputs first, then even batteries' x.
        ginT_sb = gpool.tile([128, GKT, B], F32R)
        nc.sync.dma_start(out=ginT_sb.rearrange("p k b -> p (k b)"), in_=gintp.ap())
        w1a_sb = w1pool.tile([128, GA, DFFC], F32R)
        w1a_ap = w1a.ap().rearrange("p (k f) -> p k f", k=GA)
        nc.sync.dma_start(out=w1a_sb[:, 0:9, :], in_=w1a_ap[:, 0:9, :])
        nc.sync.dma_start(out=w1a_sb[:, 9:GA, :], in_=w1a_ap[:, 9:GA, :])
        selt_sb = gpool.tile([B, BPC], F32)
        nc.sync.dma_start(out=selt_sb, in_=seltd.ap())
        b2_sb = gpool.tile([BPC, E], F32)
        nc.sync.dma_start(out=b2_sb, in_=b2d.ap())
        w2_sb = gpool.tile([128, 2, E], F32)
        nc.sync.dma_start(out=w2_sb.rearrange("p j e -> p (j e)"), in_=w2p.ap())

        # Ring B (scalar): w1 second half only, for now — the xb loads are
        # emitted after the gating ACT ops (HWDGE DMA-issue instructions
        # block the issuing engine's FIFO, and the gating tanh must not
        # queue behind ring-B transfers).
        w1b_sb = w1pool.tile([128, GB, DFFC], F32R)
        w1b_ap = w1b.ap().rearrange("p (k f) -> p k f", k=GB)
        nc.scalar.dma_start(out=w1b_sb[:, 0:8, :], in_=w1b_ap[:, 0:8, :])
        nc.scalar.dma_start(out=w1b_sb[:, 8:GB, :], in_=w1b_ap[:, 8:GB, :])

        # Fused expert weights A on ring A (sync — no compute shares it).
        A_sb = singles.tile([128, E, KT, D], BF16)
        # zero the remainder k-tile first (only rows 0:KREM carry data; the
        # combine reads the full 128 partitions of it)
        nc.gpsimd.memset(A_sb[:, :, 7, :], 0.0)
        nc.sync.dma_start(
            out=A_sb[:, :, 0:7, :],
            in_=amain.ap(),
        )
        nc.sync.dma_start(
            out=A_sb[0:KREM, :, 7, :],
            in_=arem.ap(),
        )

        # all x batteries on ring A after the expert weights (the scalar
        # ring carries only w1b so gating ACT ops never queue behind
        # DMA ring-credit waits).
        xmain_ap = xmain.ap()
        xrem_ap = xrem.ap()
        xb_tiles = [None] * BPC
        for b in range(BPC):
            xb = singles.tile([128, KT, L], BF16, tag=f"xb{b}")
            nc.sync.dma_start(
                out=xb[:, 0:7, :].rearrange("p k l -> p (k l)"),
                in_=xmain_ap[b].rearrange("p k l -> p (k l)"),
            )
            nc.sync.dma_start(out=xb[0:KREM, 7, :], in_=xrem_ap[b])
            xb_tiles[b] = xb

        # ------------- constants / scratch for gating ------------------
        ident = singles.tile([128, 128], F32)
        make_identity(nc, ident)
        # SEL[:, 0:128] selects row 0 of a [2, N] rhs; SEL[:, 128:256] row 1.
        sel2 = singles.tile([2, 256], F32)
        nc.sync.dma_start(out=sel2, in_=sel2d.ap())

        # Cross-core logits exchange buffers.  Slot 0 = my partials,
        # slot j = partials from core (me XOR j).
        gather = gpool.tile([128, E, E], F32)
        gmset = nc.gpsimd.memset(gather[:, 0, :], 0.0)

        rsem = nc.alloc_semaphore("rsem")
        lsem = nc.alloc_semaphore("lsem")

        # ------------- gating layer 1 (fp32r, d_ff-sharded) ------------
        psum_h = gps.tile([B, DFFC], F32, bufs=1)
        order = [(w1a_sb, 0, 0, 9), (w1b_sb, GA, 0, 8),
                 (w1a_sb, 0, 9, GA), (w1b_sb, GA, 8, GB)]
        n_done = 0
        for (wt, base, lo, hi) in order:
            for k in range(lo, hi):
                kt_g = base + k
                nc.tensor.matmul(
                    out=psum_h, lhsT=ginT_sb[:, kt_g, :], rhs=wt[:, k, :],
                    start=(n_done == 0), stop=(n_done == GKT - 1),
                )
                n_done += 1

        # gelu (tanh approx):  h = 0.5*x*(1 + tanh(0.79788456*(x + 0.044715*x^3)))
        g_x = gpool.tile([B, DFFC], F32)
        nc.vector.tensor_copy(out=g_x, in_=psum_h)
        g_x2 = gpool.tile([B, DFFC], F32)
        nc.vector.tensor_tensor(out=g_x2, in0=g_x, in1=g_x, op=AluOpType.mult)
        g_p = gpool.tile([B, DFFC], F32)
        nc.vector.tensor_scalar(g_p, g_x2, 0.044715, 1.0,
                                AluOpType.mult, AluOpType.add)
        g_u = gpool.tile([B, DFFC], F32)
        nc.vector.tensor_tensor(out=g_u, in0=g_x, in1=g_p, op=AluOpType.mult)
        g_t = gpool.tile([B, DFFC], F32)
        nc.scalar.activation(out=g_t, in_=g_u,
                             func=mybir.ActivationFunctionType.Tanh,
                             scale=0.7978845608028654)
        g_q = gpool.tile([B, DFFC], F32)
        nc.vector.tensor_scalar(g_q, g_t, 1.0, 0.5,
                                AluOpType.add, AluOpType.mult)
        h_sb = gpool.tile([B, DFFC], F32)
        nc.vector.tensor_tensor(out=h_sb, in0=g_x, in1=g_q, op=AluOpType.mult)

        # transpose h -> hT [128, 2, B]
        hT_sb = gpool.tile([128, 2, B], F32)
        for j in range(2):
            pst = gps.tile([128, B], F32, bufs=2, tag="pst")
            nc.tensor.transpose(
                out=pst, in_=h_sb[:, j * 128:(j + 1) * 128], identity=ident[:B, :B]
            )
            nc.vector.tensor_copy(out=hT_sb[:, j, :], in_=pst)

        # layer 2 partial logits [B, E] -> gather slot 0
        psum_l = gps.tile([B, E], F32, bufs=2, tag="pst")
        for j in range(2):
            nc.tensor.matmul(out=psum_l, lhsT=hT_sb[:, j, :], rhs=w2_sb[:, j, :],
                             start=(j == 0), stop=(j == 1))
        nc.vector.tensor_copy(out=gather[0:B, 0, :], in_=psum_l)

        # exchange: 7 single-dest broadcasts (slot j <- core me XOR j), then
        # one trigger.  The wait for the 7 peers (2 sem incs each) is
        # injected post-scheduling onto su1 (see after the TileContext) so
        # Tile's single-core scheduling sim does not deadlock on a sem it
        # cannot see incremented.
        for j in range(1, NCORES):
            rd = [None] * 8
            rd[j] = (0, j)
            nc.gpsimd.remote_dma_broadcast(
                gather[:, j, :], gather[:, 0, :],
                remote_sem=rsem, local_sem=lsem, rdests=rd,
            )
        trig = nc.gpsimd.trigger_dma(count=None)
        add_dep_helper(trig.ins, gmset.ins, sync=True,
                       reason="src tile fully initialized before exchange")

        # Fire-and-forget tiny AllReduce AFTER the exchange trigger in the
        # gpsimd stream (its ncfw barrier blocks gpsimd for ~40us).  Its only
        # job: registering this NEFF with the collectives runtime so the 8
        # cores are gang-launched — without it, per-core launch skew reaches
        # milliseconds and the logits exchange stalls on it.  Result unused;
        # the barrier+reduce run on TOPSP firmware concurrent with the main
        # fused phase.
        cc_src = gpool.tile([1, 8], F32, tag="cc_src")
        nc.gpsimd.memset(cc_src, 0.0)
        cc_in = dpool.tile([1, 8], F32)
        cc_dma = nc.gpsimd.dma_start(out=cc_in, in_=cc_src)
        add_dep_helper(cc_dma.ins, trig.ins, sync=False,
                       reason="keep collective path after exchange trigger")
        cc_out = dpool.tile([1, 8], F32, addr_space="Shared")
        nc.gpsimd.collective_compute(
            "AllReduce", AluOpType.add,
            replica_groups=[list(range(NCORES))],
            ins=[cc_in], outs=[cc_out],
        )
        s4 = gpool.tile([128, 4, E], F32)
        su1 = nc.vector.tensor_tensor(
            out=s4.rearrange("p s e -> p (s e)"),
            in0=gather[:, 0:4, :].rearrange("p s e -> p (s e)"),
            in1=gather[:, 4:8, :].rearrange("p s e -> p (s e)"),
            op=AluOpType.add,
        )
        add_dep_helper(su1.ins, trig.ins, sync=False,
                       reason="sum scheduled after exchange trigger")
        su1_ins = su1.ins

        # PE warm-up: ~4us of junk matmuls starting right after the
        # exchange wait passes, so the HAM clock-gate is at K=8/8 when the
        # fused matmuls begin (the PE idles while waiting for peers and
        # gets re-throttled to 1.2 GHz otherwise).  Results overwrite the
        # dead gating PSUM tile.
        for j in range(16):
            jmm = nc.tensor.matmul(
                out=psum_h, lhsT=ginT_sb[:, j, :], rhs=w1a_sb[:, j, :],
                start=True, stop=True,
            )
            if j == 0:
                add_dep_helper(jmm.ins, su1.ins, sync=True,
                               reason="warm-up matmuls start at exchange completion")
        s2 = gpool.tile([128, 2, E], F32)
        nc.vector.tensor_tensor(
            out=s2.rearrange("p s e -> p (s e)"),
            in0=s4[:, 0:2, :].rearrange("p s e -> p (s e)"),
            in1=s4[:, 2:4, :].rearrange("p s e -> p (s e)"),
            op=AluOpType.add,
        )
        logits_all = gpool.tile([B, E], F32)
        nc.vector.tensor_tensor(out=logits_all, in0=s2[0:B, 0, :],
                                in1=s2[0:B, 1, :], op=AluOpType.add)

        # select my 8 batteries (one-hot matmul), add gate_b2
        psum_sel = gps.tile([BPC, E], F32, bufs=2, tag="pst")
        nc.tensor.matmul(out=psum_sel, lhsT=selt_sb, rhs=logits_all,
                         start=True, stop=True)
        logits_my = gpool.tile([BPC, E], F32)
        nc.vector.tensor_tensor(out=logits_my, in0=psum_sel, in1=b2_sb,
                                op=AluOpType.add)

        # top-2 gates: sorted values + indices, softmax renorm on top-2
        sorted8 = gpool.tile([BPC, E], F32)
        sidx = gpool.tile([BPC, E], U32)
        nc.vector.max(out=sorted8, in_=logits_my)
        nc.vector.max_index(out=sidx, in_max=sorted8, in_values=logits_my)
        negmax = gpool.tile([BPC, 1], F32)
        nc.vector.tensor_scalar_mul(negmax, sorted8[:, 0:1], -1.0)
        q = gpool.tile([BPC, E], F32)
        nc.scalar.activation(out=q, in_=sorted8,
                             func=mybir.ActivationFunctionType.Exp,
                             bias=negmax, scale=1.0)
        zsum = gpool.tile([BPC, 1], F32)
        nc.vector.reduce_sum(out=zsum, in_=q, axis=mybir.AxisListType.X)
        t12 = gpool.tile([BPC, 1], F32)
        nc.vector.tensor_tensor(out=t12, in0=q[:, 0:1], in1=q[:, 1:2],
                                op=AluOpType.add)
        den = gpool.tile([BPC, 1], F32)
        nc.vector.scalar_tensor_tensor(out=den, in0=zsum, scalar=EPS, in1=t12,
                                       op0=AluOpType.mult, op1=AluOpType.add)
        rden = gpool.tile([BPC, 1], F32)
        nc.vector.reciprocal(out=rden, in_=den)
        g12 = gpool.tile([BPC, 2], F32)
        nc.vector.tensor_scalar_mul(g12, q[:, 0:2], rden)

        # broadcast g1/g2 of each battery to all 128 partitions via PE:
        # transpose g12 -> [2, BPC], then ones-row matmuls.
        psum_tr = gps.tile([2, BPC], F32, bufs=2, tag="pst")
        nc.tensor.transpose(out=psum_tr, in_=g12, identity=ident[:BPC, :BPC])
        trs = gpool.tile([2, BPC], F32)
        nc.vector.tensor_copy(out=trs, in_=psum_tr)
        bcG = []
        for r in range(2):
            pbc = gps.tile([128, BPC], F32, bufs=2, tag="pbc")
            nc.tensor.matmul(out=pbc, lhsT=sel2[:, r * 128:(r + 1) * 128],
                             rhs=trs, start=True, stop=True)
            g_sb = gpool.tile([128, BPC], F32, tag=f"bcg{r}")
            nc.vector.tensor_copy(out=g_sb, in_=pbc)
            bcG.append(g_sb)

        gps_ctx.close()
        w1_ctx.close()

        # ------------- main fused phase --------------------------------
        mps = ctx.enter_context(tc.tile_pool(name="mpsum", bufs=2, space="PSUM"))
        wbpool = ctx.enter_context(tc.tile_pool(name="wbs", bufs=2))
        scpool = ctx.enter_context(tc.tile_pool(name="scratch", bufs=2))
        opool = ctx.enter_context(tc.tile_pool(name="outs", bufs=3))

        def _vload(eng, ap, name):
            reg = eng.alloc_register(name)
            eng.reg_load(reg, ap)
            val = eng.snap(reg, donate=True)
            return nc.s_assert_within(val, 0, E - 1, skip_runtime_assert=True)

        def combine(b, pieces=2):
            """wb = g1*A_e1 + g2*A_e2 for battery b.

            t2 = g2*A_e2 on ACT (activation scale), t1 = g1*A_e1 and the
            add on DVE (both run in DVE high-perf modes).  `pieces` splits
            the kt range so the first fused matmuls can start early.
            """
            rv1 = _vload(nc.vector, sidx[b:b + 1, 0:1], f"e1_{b}")
            rv2 = _vload(nc.scalar, sidx[b:b + 1, 1:2], f"e2_{b}")
            wb = wbpool.tile([128, KT, D], BF16)
            w = KT // pieces
            for h in range(pieces):
                kts = slice(h * w, (h + 1) * w)
                t2 = scpool.tile([128, w, D], BF16, tag=f"t2_{pieces}")
                nc.scalar.activation(
                    out=t2.rearrange("p k d -> p (k d)"),
                    in_=A_sb[:, ds(rv2, 1), kts, :].rearrange("p o k d -> p (o k d)"),
                    func=mybir.ActivationFunctionType.Copy,
                    scale=bcG[1][:, b:b + 1],
                )
                t1 = scpool.tile([128, w, D], BF16, tag=f"t1_{pieces}")
                nc.vector.tensor_scalar_mul(
                    t1.rearrange("p k d -> p (k d)"),
                    A_sb[:, ds(rv1, 1), kts, :].rearrange("p o k d -> p (o k d)"),
                    bcG[0][:, b:b + 1],
                )
                nc.vector.tensor_tensor(
                    out=wb[:, kts, :].rearrange("p k d -> p (k d)"),
                    in0=t1.rearrange("p k d -> p (k d)"),
                    in1=t2.rearrange("p k d -> p (k d)"),
                    op=AluOpType.add,
                )
            return wb

        def battery(b, wb):
            xb = xb_tiles[b]
            pm = mps.tile([128, MT, D], F32, tag="mp")
            for kt in range(KT):
                np_ = KREM if kt == 7 else 128
                for m in range(MT):
                    nc.tensor.matmul(
                        out=pm[:, m, :],
                        lhsT=xb[0:np_, kt, ts(m, 128)],
                        rhs=wb[0:np_, kt, :],
                        start=(kt == 0), stop=(kt == KT - 1),
                    )
            osb = opool.tile([128, MT, D], BF16, tag="osb")
            for m in range(MT):
                if m % 2 == 0:
                    nc.vector.tensor_copy(out=osb[:, m, :], in_=pm[:, m, :])
                else:
                    nc.scalar.activation(out=osb[:, m, :], in_=pm[:, m, :],
                                         func=mybir.ActivationFunctionType.Copy)
            return nc.sync.dma_start(
                out=outd.ap()[b].rearrange("p m d -> p (m d)"),
                in_=osb.rearrange("p m d -> p (m d)"),
            )

        wbs = {0: combine(0, pieces=4), 1: combine(1, pieces=4)}
        for b in range(BPC):
            battery(b, wbs.pop(b))
            if b + 2 < BPC:
                wbs[b + 2] = combine(b + 2)

    # After the TileContext (which ends with drain + all-engine barrier):
    # reset the exchange semaphores so a second execution of this NEFF
    # starts from zero.  The lsem wait proves all 7 sends retired (16 local
    # increments each) before the clear.
    nc.gpsimd.wait_ge(lsem, 112)
    nc.gpsimd.drain()
    nc.all_engine_barrier()
    for s in (rsem, lsem):
        nc.gpsimd.dma_reset(range(s.num, s.num + 1))
        nc.gpsimd.sem_clear(range(s.num, s.num + 1))
    nc.all_engine_barrier()

    # Injected post-scheduling: su1 must wait for the 7 peers' partial
    # logits to land (2 rsem increments per peer).  Added here, after the
    # TileContext has scheduled, so the single-core scheduling sim never
    # blocks on a semaphore only remote cores increment.
    su1_ins.sync_info.on_wait.append(
        mybir.SyncWait(sync_type="semaphore", id=rsem.num,
                       wait_mode="sem-ge-imm", wait_value=14,
                       ant_name=rsem.name)
    )


def make_nc():
    from concourse import bacc
    nc = bacc.Bacc("TRN2", target_bir_lowering=False, debug=False,
                   num_devices=NCORES)
    build_program(nc)
    nc.finalize()
    return nc


def prep_inputs(cycle_curve_data, cycle_numbers, DKP_embeddings,
                gate_W1, gate_b1, gate_W2, gate_b2,
                expert_W, expert_b, gen_W, gen_b):
    """Host-side layout prep (reshape/pad/cast/weight-fold). Returns in_maps."""
    f32 = np.float32
    bf16 = ml_dtypes.bfloat16

    # fused expert weights A_e = gen_W + expert_W[e]; ones-row bias.
    A = np.empty((E, K, D), dtype=f32)
    A[:, :CF, :] = np.asarray(expert_W, dtype=f32) + np.asarray(gen_W, dtype=f32)
    A[:, CF, :] = np.asarray(expert_b, dtype=f32) + np.asarray(gen_b, dtype=f32)
    Abf = A.astype(bf16)
    amain = np.ascontiguousarray(
        Abf[:, :896, :].reshape(E, 7, 128, D).transpose(2, 0, 1, 3))
    arem = np.ascontiguousarray(Abf[:, 896:K, :].transpose(1, 0, 2))

    # x transposed with ones-row, partition-major.
    x = np.asarray(cycle_curve_data, dtype=f32).reshape(B, L, CF)
    xT = np.empty((B, K, L), dtype=bf16)
    xT[:, :CF, :] = x.transpose(0, 2, 1).astype(bf16)
    xT[:, CF, :] = np.asarray(1.0, dtype=bf16)
    xmain = np.ascontiguousarray(
        xT[:, :896, :].reshape(B, 7, 128, L).transpose(0, 2, 1, 3))
    xrem = np.ascontiguousarray(xT[:, 896:K, :])

    # gating input, partition-major [128, 33*64].
    g = np.zeros((GK, B), dtype=f32)
    g[:DLLM, :] = np.asarray(DKP_embeddings, dtype=f32).T
    g[DLLM, :] = np.asarray(cycle_numbers, dtype=f32)[:, 0]
    g[DLLM + 1, :] = 1.0
    gintp = np.ascontiguousarray(
        g.reshape(GKT, 128, B).transpose(1, 0, 2).reshape(128, GKT * B))

    W1p = np.zeros((GK, DFF), dtype=f32)
    W1p[:DLLM + 1, :] = np.asarray(gate_W1, dtype=f32)
    W1p[DLLM + 1, :] = np.asarray(gate_b1, dtype=f32)

    w2 = np.asarray(gate_W2, dtype=f32)
    b2rep = np.tile(np.asarray(gate_b2, dtype=f32).reshape(1, E), (BPC, 1))

    in_maps = []
    for c in range(NCORES):
        chunk = W1p[:, c * DFFC:(c + 1) * DFFC]
        w1pm = chunk.reshape(GKT, 128, DFFC).transpose(1, 0, 2)
        w1a = np.ascontiguousarray(w1pm[:, :GA, :].reshape(128, GA * DFFC))
        w1b = np.ascontiguousarray(w1pm[:, GA:, :].reshape(128, GB * DFFC))
        w2pm = np.ascontiguousarray(
            w2[c * DFFC:(c + 1) * DFFC, :].reshape(2, 128, E)
            .transpose(1, 0, 2).reshape(128, 2 * E))
        sel = np.zeros((B, BPC), dtype=f32)
        for i in range(BPC):
            sel[c * BPC + i, i] = 1.0
        sel2 = np.zeros((2, 256), dtype=f32)
        sel2[0, 0:128] = 1.0
        sel2[1, 128:256] = 1.0
        in_maps.append({
            "xmain": np.ascontiguousarray(xmain[c * BPC:(c + 1) * BPC]),
            "xrem": np.ascontiguousarray(xrem[c * BPC:(c + 1) * BPC]),
            "amain": amain,
            "arem": arem,
            "gintp": gintp,
            "w1a": w1a,
            "w1b": w1b,
            "w2p": w2pm,
            "selt": sel,
            "b2rep": b2rep,
            "sel2": sel2,
        })
    return in_maps


_CACHED = {}


def run(inputs, trace=False, tmpdir=None):
    """Run on the 8 NeuronCores; returns (full_output, BassKernelResults)."""
    from concourse import bass_utils
    in_maps = prep_inputs(**inputs)
    nc = _CACHED.get("nc")
    if nc is None:
        nc = make_nc()
        _CACHED["nc"] = nc
    res = bass_utils.run_bass_kernel_spmd(
        nc, in_maps, core_ids=list(range(NCORES)), trace=trace, tmpdir=tmpdir
    )
    outs = [np.asarray(r["out"]) for r in res.results]
    full = np.concatenate(outs, axis=0)          # [B, 128, MT, D] bf16
    full = full.transpose(0, 2, 1, 3).reshape(B, L, D).astype(np.float32)
    return full, res


def kernel(**inputs):
    full, _ = run(inputs, trace=False)
    return full


# revision 41
# speedup vs baseline: 46.7416x; 1.1382x over previous
"""Trainium2 Bass kernel for FlattenIntraCycleMoELayer (top-2 MoE + general path).

Strategy (v2):
  - Data-parallel over B (8 batteries per core).
  - gen_W is folded into each expert on host (gates sum to 1), so each
    battery is ONE fused matmul set: out = x @ (g1*A_e1 + g2*A_e2),
    A_e = gen_W + expert_W[e], bias folded via an appended ones-row.
  - Gating layer-1 is d_ff-sharded (fp32r matmul, 1 cyc/row); partial
    logits are exchanged core-to-core with remote_dma_broadcast (7
    single-dest sends into per-peer slots) instead of the ncfw
    AllReduce, then tree-summed locally.
  - Unpadded K=901 (7 full k-tiles + 5-row remainder), partition-major
    host layouts so every big DMA is contiguous per partition.
  - Output written bf16 (host casts to f32).
  - Main loop: kt-major matmuls per battery (dense PE stream), combine
    one battery ahead (t1 on DVE, t2 on ACT, add on DVE), evictions
    split DVE/ACT, PSUM double-buffered.

Host-side prep only reshapes/pads/casts/re-parametrizes weights
(elementwise adds of gen_W/gen_b into expert weights); all model math
runs on device.
"""

import numpy as np
import ml_dtypes


def _ensure_import_path():
    try:
        import concourse  # noqa: F401
    except ImportError:
        import sys
        for p in ("/opt/trn_rl_repo", "/root/.axon_site/_ro/trn_rl_repo"):
            if p not in sys.path:
                sys.path.insert(0, p)
        import concourse  # noqa: F401


_ensure_import_path()

import concourse.bass as bass  # noqa: E402
import concourse.tile as tile  # noqa: E402
from concourse import mybir  # noqa: E402
from concourse.bass import ds, ts  # noqa: E402
from concourse.alu_op_type import AluOpType  # noqa: E402
from concourse.masks import make_identity  # noqa: E402
from concourse.tile import add_dep_helper  # noqa: E402

BF16 = mybir.dt.bfloat16
F32 = mybir.dt.float32
F32R = mybir.dt.float32r
F16 = mybir.dt.float16
U32 = mybir.dt.uint32

# Problem shape constants (hardcoded per contest rules).
B, L, C, F = 64, 512, 3, 300
CF = C * F              # 900
K = CF + 1              # 901 contraction rows (data + ones row for bias)
KT = 8                  # k-tiles: 7 full + 1 remainder
KREM = K - 7 * 128      # 5 rows in the last k-tile
D = 512                 # d_model
E = 8                   # experts
NCORES = 8
BPC = B // NCORES       # 8 batteries per core
DLLM = 4096
GK = 4224               # padded gating contraction = 33*128
GKT = GK // 128         # 33
GA = 17                 # w1 k-tiles on ring A (sync)
GB = GKT - GA           # 16 k-tiles on ring B (scalar)
DFF = 2048
DFFC = DFF // NCORES    # 256 per-core d_ff chunk
EPS = 1e-9
MT = L // 128           # 4 m-tiles per battery


def build_program(nc):
    from contextlib import ExitStack

    xmain = nc.dram_tensor("xmain", [BPC, 128, 7, L], BF16, kind="ExternalInput")
    xrem = nc.dram_tensor("xrem", [BPC, KREM, L], BF16, kind="ExternalInput")
    amain = nc.dram_tensor("amain", [128, E, 7, D], BF16, kind="ExternalInput")
    arem = nc.dram_tensor("arem", [KREM, E, D], BF16, kind="ExternalInput")
    gintp = nc.dram_tensor("gintp", [128, GKT * B], F16, kind="ExternalInput")
    w1a = nc.dram_tensor("w1a", [128, GA * DFFC], F16, kind="ExternalInput")
    w1b = nc.dram_tensor("w1b", [128, GB * DFFC], F16, kind="ExternalInput")
    w2p = nc.dram_tensor("w2p", [128, 2 * E], F32, kind="ExternalInput")
    seltd = nc.dram_tensor("selt", [B, BPC], F32, kind="ExternalInput")
    b2d = nc.dram_tensor("b2rep", [BPC, E], F32, kind="ExternalInput")
    sel2d = nc.dram_tensor("sel2", [2, 256], F32, kind="ExternalInput")
    outd = nc.dram_tensor("out", [BPC, 128, MT, D], BF16, kind="ExternalOutput")

    with tile.TileContext(nc) as tc, ExitStack() as ctx:
        singles = ctx.enter_context(tc.tile_pool(name="singles", bufs=1))
        gpool = ctx.enter_context(tc.tile_pool(name="gate", bufs=1))
        dpool = ctx.enter_context(tc.tile_pool(name="dram", bufs=1, space="DRAM"))


        w1_ctx = ExitStack()
        w1pool = w1_ctx.enter_context(tc.tile_pool(name="w1s", bufs=1))
        gps_ctx = ExitStack()
        gps = gps_ctx.enter_context(tc.tile_pool(name="gpsum", bufs=1, space="PSUM"))

        # ------------- DMAs: ring A = nc.sync, ring B = nc.scalar -------
        # Ring A: gating inputs first, then even batteries' x.
        ginT_sb = gpool.tile([128, GKT, B], F16)
        nc.sync.dma_start(out=ginT_sb.rearrange("p k b -> p (k b)"), in_=gintp.ap())
        w1a_sb = w1pool.tile([128, GA, DFFC], F16)
        w1a_ap = w1a.ap().rearrange("p (k f) -> p k f", k=GA)
        nc.sync.dma_start(out=w1a_sb[:, 0:9, :], in_=w1a_ap[:, 0:9, :])
        nc.sync.dma_start(out=w1a_sb[:, 9:GA, :], in_=w1a_ap[:, 9:GA, :])
        selt_sb = gpool.tile([B, BPC], F32)
        nc.sync.dma_start(out=selt_sb, in_=seltd.ap())
        b2_sb = gpool.tile([BPC, E], F32)
        nc.sync.dma_start(out=b2_sb, in_=b2d.ap())
        w2_sb = gpool.tile([128, 2, E], F32)
        nc.sync.dma_start(out=w2_sb.rearrange("p j e -> p (j e)"), in_=w2p.ap())

        # Ring B (scalar): w1 second half only, for now — the xb loads are
        # emitted after the gating ACT ops (HWDGE DMA-issue instructions
        # block the issuing engine's FIFO, and the gating tanh must not
        # queue behind ring-B transfers).
        w1b_sb = w1pool.tile([128, GB, DFFC], F16)
        w1b_ap = w1b.ap().rearrange("p (k f) -> p k f", k=GB)
        nc.scalar.dma_start(out=w1b_sb[:, 0:8, :], in_=w1b_ap[:, 0:8, :])
        nc.scalar.dma_start(out=w1b_sb[:, 8:GB, :], in_=w1b_ap[:, 8:GB, :])

        # Fused expert weights A on ring A (sync — no compute shares it).
        A_sb = singles.tile([128, E, KT, D], BF16)
        # zero the remainder k-tile first (only rows 0:KREM carry data; the
        # combine reads the full 128 partitions of it)
        nc.gpsimd.memset(A_sb[:, :, 7, :], 0.0)
        nc.sync.dma_start(
            out=A_sb[:, :, 0:7, :],
            in_=amain.ap(),
        )
        nc.sync.dma_start(
            out=A_sb[0:KREM, :, 7, :],
            in_=arem.ap(),
        )

        # all x batteries on ring A after the expert weights (the scalar
        # ring carries only w1b so gating ACT ops never queue behind
        # DMA ring-credit waits).
        xmain_ap = xmain.ap()
        xrem_ap = xrem.ap()
        xb_tiles = [None] * BPC
        for b in range(BPC):
            xb = singles.tile([128, KT, L], BF16, tag=f"xb{b}")
            nc.sync.dma_start(
                out=xb[:, 0:7, :].rearrange("p k l -> p (k l)"),
                in_=xmain_ap[b].rearrange("p k l -> p (k l)"),
            )
            nc.sync.dma_start(out=xb[0:KREM, 7, :], in_=xrem_ap[b])
            xb_tiles[b] = xb

        # ------------- constants / scratch for gating ------------------
        ident = singles.tile([128, 128], F32)
        make_identity(nc, ident)
        # SEL[:, 0:128] selects row 0 of a [2, N] rhs; SEL[:, 128:256] row 1.
        sel2 = singles.tile([2, 256], F32)
        nc.sync.dma_start(out=sel2, in_=sel2d.ap())

        # Cross-core logits exchange buffers.  Slot 0 = my partials,
        # slot j = partials from core (me XOR j).
        gather = gpool.tile([128, E, E], F32)
        gmset = nc.gpsimd.memset(gather[:, 0, :], 0.0)

        rsem = nc.alloc_semaphore("rsem")
        lsem = nc.alloc_semaphore("lsem")

        # ------------- gating layer 1 (fp32r, d_ff-sharded) ------------
        psum_h = gps.tile([B, DFFC], F32, bufs=1)
        order = [(w1a_sb, 0, 0, 9), (w1b_sb, GA, 0, 8),
                 (w1a_sb, 0, 9, GA), (w1b_sb, GA, 8, GB)]
        n_done = 0
        for (wt, base, lo, hi) in order:
            for k in range(lo, hi):
                kt_g = base + k
                nc.tensor.matmul(
                    out=psum_h, lhsT=ginT_sb[:, kt_g, :], rhs=wt[:, k, :],
                    start=(n_done == 0), stop=(n_done == GKT - 1),
                )
                n_done += 1

        # gelu (tanh approx):  h = 0.5*x*(1 + tanh(0.79788456*(x + 0.044715*x^3)))
        g_x = gpool.tile([B, DFFC], F32)
        nc.vector.tensor_copy(out=g_x, in_=psum_h)
        g_x2 = gpool.tile([B, DFFC], F32)
        nc.vector.tensor_tensor(out=g_x2, in0=g_x, in1=g_x, op=AluOpType.mult)
        g_p = gpool.tile([B, DFFC], F32)
        nc.vector.tensor_scalar(g_p, g_x2, 0.044715, 1.0,
                                AluOpType.mult, AluOpType.add)
        g_u = gpool.tile([B, DFFC], F32)
        nc.vector.tensor_tensor(out=g_u, in0=g_x, in1=g_p, op=AluOpType.mult)
        g_t = gpool.tile([B, DFFC], F32)
        nc.scalar.activation(out=g_t, in_=g_u,
                             func=mybir.ActivationFunctionType.Tanh,
                             scale=0.7978845608028654)
        g_q = gpool.tile([B, DFFC], F32)
        nc.vector.tensor_scalar(g_q, g_t, 1.0, 0.5,
                                AluOpType.add, AluOpType.mult)
        h_sb = gpool.tile([B, DFFC], F32)
        nc.vector.tensor_tensor(out=h_sb, in0=g_x, in1=g_q, op=AluOpType.mult)

        # transpose h -> hT [128, 2, B]
        hT_sb = gpool.tile([128, 2, B], F32)
        for j in range(2):
            pst = gps.tile([128, B], F32, bufs=2, tag="pst")
            nc.tensor.transpose(
                out=pst, in_=h_sb[:, j * 128:(j + 1) * 128], identity=ident[:B, :B]
            )
            nc.vector.tensor_copy(out=hT_sb[:, j, :], in_=pst)

        # layer 2 partial logits [B, E] -> gather slot 0
        psum_l = gps.tile([B, E], F32, bufs=2, tag="pst")
        for j in range(2):
            nc.tensor.matmul(out=psum_l, lhsT=hT_sb[:, j, :], rhs=w2_sb[:, j, :],
                             start=(j == 0), stop=(j == 1))
        nc.vector.tensor_copy(out=gather[0:B, 0, :], in_=psum_l)

        # exchange: 7 single-dest broadcasts (slot j <- core me XOR j), then
        # one trigger.  The wait for the 7 peers (2 sem incs each) is
        # injected post-scheduling onto su1 (see after the TileContext) so
        # Tile's single-core scheduling sim does not deadlock on a sem it
        # cannot see incremented.
        for j in range(1, NCORES):
            rd = [None] * 8
            rd[j] = (0, j)
            nc.gpsimd.remote_dma_broadcast(
                gather[:, j, :], gather[:, 0, :],
                remote_sem=rsem, local_sem=lsem, rdests=rd,
            )
        trig = nc.gpsimd.trigger_dma(count=None)
        add_dep_helper(trig.ins, gmset.ins, sync=True,
                       reason="src tile fully initialized before exchange")

        # Fire-and-forget tiny AllReduce AFTER the exchange trigger in the
        # gpsimd stream (its ncfw barrier blocks gpsimd for ~40us).  Its only
        # job: registering this NEFF with the collectives runtime so the 8
        # cores are gang-launched — without it, per-core launch skew reaches
        # milliseconds and the logits exchange stalls on it.  Result unused;
        # the barrier+reduce run on TOPSP firmware concurrent with the main
        # fused phase.
        cc_src = gpool.tile([1, 8], F32, tag="cc_src")
        nc.gpsimd.memset(cc_src, 0.0)
        cc_in = dpool.tile([1, 8], F32)
        cc_dma = nc.gpsimd.dma_start(out=cc_in, in_=cc_src)
        add_dep_helper(cc_dma.ins, trig.ins, sync=False,
                       reason="keep collective path after exchange trigger")
        cc_out = dpool.tile([1, 8], F32, addr_space="Shared")
        nc.gpsimd.collective_compute(
            "AllReduce", AluOpType.add,
            replica_groups=[list(range(NCORES))],
            ins=[cc_in], outs=[cc_out],
        )
        s4 = gpool.tile([128, 4, E], F32)
        su1 = nc.vector.tensor_tensor(
            out=s4.rearrange("p s e -> p (s e)"),
            in0=gather[:, 0:4, :].rearrange("p s e -> p (s e)"),
            in1=gather[:, 4:8, :].rearrange("p s e -> p (s e)"),
            op=AluOpType.add,
        )
        add_dep_helper(su1.ins, trig.ins, sync=False,
                       reason="sum scheduled after exchange trigger")
        su1_ins = su1.ins

        # PE warm-up: ~4us of junk matmuls starting right after the
        # exchange wait passes, so the HAM clock-gate is at K=8/8 when the
        # fused matmuls begin (the PE idles while waiting for peers and
        # gets re-throttled to 1.2 GHz otherwise).  Results overwrite the
        # dead gating PSUM tile.
        for j in range(16):
            jmm = nc.tensor.matmul(
                out=psum_h, lhsT=ginT_sb[:, j, :], rhs=w1a_sb[:, j, :],
                start=True, stop=True,
            )
            if j == 0:
                add_dep_helper(jmm.ins, su1.ins, sync=True,
                               reason="warm-up matmuls start at exchange completion")
        s2 = gpool.tile([128, 2, E], F32)
        nc.vector.tensor_tensor(
            out=s2.rearrange("p s e -> p (s e)"),
            in0=s4[:, 0:2, :].rearrange("p s e -> p (s e)"),
            in1=s4[:, 2:4, :].rearrange("p s e -> p (s e)"),
            op=AluOpType.add,
        )
        logits_all = gpool.tile([B, E], F32)
        nc.vector.tensor_tensor(out=logits_all, in0=s2[0:B, 0, :],
                                in1=s2[0:B, 1, :], op=AluOpType.add)

        # select my 8 batteries (one-hot matmul), add gate_b2
        psum_sel = gps.tile([BPC, E], F32, bufs=2, tag="pst")
        nc.tensor.matmul(out=psum_sel, lhsT=selt_sb, rhs=logits_all,
                         start=True, stop=True)
        logits_my = gpool.tile([BPC, E], F32)
        nc.vector.tensor_tensor(out=logits_my, in0=psum_sel, in1=b2_sb,
                                op=AluOpType.add)

        # top-2 gates: sorted values + indices, softmax renorm on top-2
        sorted8 = gpool.tile([BPC, E], F32)
        sidx = gpool.tile([BPC, E], U32)
        nc.vector.max(out=sorted8, in_=logits_my)
        nc.vector.max_index(out=sidx, in_max=sorted8, in_values=logits_my)
        negmax = gpool.tile([BPC, 1], F32)
        nc.vector.tensor_scalar_mul(negmax, sorted8[:, 0:1], -1.0)
        q = gpool.tile([BPC, E], F32)
        nc.scalar.activation(out=q, in_=sorted8,
                             func=mybir.ActivationFunctionType.Exp,
                             bias=negmax, scale=1.0)
        zsum = gpool.tile([BPC, 1], F32)
        nc.vector.reduce_sum(out=zsum, in_=q, axis=mybir.AxisListType.X)
        t12 = gpool.tile([BPC, 1], F32)
        nc.vector.tensor_tensor(out=t12, in0=q[:, 0:1], in1=q[:, 1:2],
                                op=AluOpType.add)
        den = gpool.tile([BPC, 1], F32)
        nc.vector.scalar_tensor_tensor(out=den, in0=zsum, scalar=EPS, in1=t12,
                                       op0=AluOpType.mult, op1=AluOpType.add)
        rden = gpool.tile([BPC, 1], F32)
        nc.vector.reciprocal(out=rden, in_=den)
        g12 = gpool.tile([BPC, 2], F32)
        nc.vector.tensor_scalar_mul(g12, q[:, 0:2], rden)

        # broadcast g1/g2 of each battery to all 128 partitions via PE:
        # transpose g12 -> [2, BPC], then ones-row matmuls.
        psum_tr = gps.tile([2, BPC], F32, bufs=2, tag="pst")
        nc.tensor.transpose(out=psum_tr, in_=g12, identity=ident[:BPC, :BPC])
        trs = gpool.tile([2, BPC], F32)
        nc.vector.tensor_copy(out=trs, in_=psum_tr)
        bcG = []
        for r in range(2):
            pbc = gps.tile([128, BPC], F32, bufs=2, tag="pbc")
            nc.tensor.matmul(out=pbc, lhsT=sel2[:, r * 128:(r + 1) * 128],
                             rhs=trs, start=True, stop=True)
            g_sb = gpool.tile([128, BPC], F32, tag=f"bcg{r}")
            nc.vector.tensor_copy(out=g_sb, in_=pbc)
            bcG.append(g_sb)

        gps_ctx.close()
        w1_ctx.close()

        # ------------- main fused phase --------------------------------
        mps = ctx.enter_context(tc.tile_pool(name="mpsum", bufs=2, space="PSUM"))
        wbpool = ctx.enter_context(tc.tile_pool(name="wbs", bufs=2))
        scpool = ctx.enter_context(tc.tile_pool(name="scratch", bufs=2))
        opool = ctx.enter_context(tc.tile_pool(name="outs", bufs=3))

        def _vload(eng, ap, name):
            reg = eng.alloc_register(name)
            eng.reg_load(reg, ap)
            val = eng.snap(reg, donate=True)
            return nc.s_assert_within(val, 0, E - 1, skip_runtime_assert=True)

        def combine(b, pieces=2):
            """wb = g1*A_e1 + g2*A_e2 for battery b.

            t2 = g2*A_e2 on ACT (activation scale), t1 = g1*A_e1 and the
            add on DVE (both run in DVE high-perf modes).  `pieces` splits
            the kt range so the first fused matmuls can start early.
            """
            rv1 = _vload(nc.vector, sidx[b:b + 1, 0:1], f"e1_{b}")
            rv2 = _vload(nc.scalar, sidx[b:b + 1, 1:2], f"e2_{b}")
            wb = wbpool.tile([128, KT, D], BF16)
            w = KT // pieces
            for h in range(pieces):
                kts = slice(h * w, (h + 1) * w)
                t2 = scpool.tile([128, w, D], BF16, tag=f"t2_{pieces}")
                nc.scalar.activation(
                    out=t2.rearrange("p k d -> p (k d)"),
                    in_=A_sb[:, ds(rv2, 1), kts, :].rearrange("p o k d -> p (o k d)"),
                    func=mybir.ActivationFunctionType.Copy,
                    scale=bcG[1][:, b:b + 1],
                )
                t1 = scpool.tile([128, w, D], BF16, tag=f"t1_{pieces}")
                nc.vector.tensor_scalar_mul(
                    t1.rearrange("p k d -> p (k d)"),
                    A_sb[:, ds(rv1, 1), kts, :].rearrange("p o k d -> p (o k d)"),
                    bcG[0][:, b:b + 1],
                )
                nc.vector.tensor_tensor(
                    out=wb[:, kts, :].rearrange("p k d -> p (k d)"),
                    in0=t1.rearrange("p k d -> p (k d)"),
                    in1=t2.rearrange("p k d -> p (k d)"),
                    op=AluOpType.add,
                )
            return wb

        def battery(b, wb):
            xb = xb_tiles[b]
            pm = mps.tile([128, MT, D], F32, tag="mp")
            for kt in range(KT):
                np_ = KREM if kt == 7 else 128
                for m in range(MT):
                    nc.tensor.matmul(
                        out=pm[:, m, :],
                        lhsT=xb[0:np_, kt, ts(m, 128)],
                        rhs=wb[0:np_, kt, :],
                        start=(kt == 0), stop=(kt == KT - 1),
                    )
            osb = opool.tile([128, MT, D], BF16, tag="osb")
            for m in range(MT):
                if m % 2 == 0:
                    nc.vector.tensor_copy(out=osb[:, m, :], in_=pm[:, m, :])
                else:
                    nc.scalar.activation(out=osb[:, m, :], in_=pm[:, m, :],
                                         func=mybir.ActivationFunctionType.Copy)
            return nc.sync.dma_start(
                out=outd.ap()[b].rearrange("p m d -> p (m d)"),
                in_=osb.rearrange("p m d -> p (m d)"),
            )

        wbs = {0: combine(0, pieces=4), 1: combine(1, pieces=4)}
        for b in range(BPC):
            battery(b, wbs.pop(b))
            if b + 2 < BPC:
                wbs[b + 2] = combine(b + 2)

    # After the TileContext (which ends with drain + all-engine barrier):
    # reset the exchange semaphores so a second execution of this NEFF
    # starts from zero.  The lsem wait proves all 7 sends retired (16 local
    # increments each) before the clear.
    nc.gpsimd.wait_ge(lsem, 112)
    nc.gpsimd.drain()
    nc.all_engine_barrier()
    for s in (rsem, lsem):
        nc.gpsimd.dma_reset(range(s.num, s.num + 1))
        nc.gpsimd.sem_clear(range(s.num, s.num + 1))
    nc.all_engine_barrier()

    # Injected post-scheduling: su1 must wait for the 7 peers' partial
    # logits to land (2 rsem increments per peer).  Added here, after the
    # TileContext has scheduled, so the single-core scheduling sim never
    # blocks on a semaphore only remote cores increment.
    su1_ins.sync_info.on_wait.append(
        mybir.SyncWait(sync_type="semaphore", id=rsem.num,
                       wait_mode="sem-ge-imm", wait_value=14,
                       ant_name=rsem.name)
    )


def make_nc():
    from concourse import bacc
    nc = bacc.Bacc("TRN2", target_bir_lowering=False, debug=False,
                   num_devices=NCORES)
    build_program(nc)
    nc.finalize()
    return nc


def prep_inputs(cycle_curve_data, cycle_numbers, DKP_embeddings,
                gate_W1, gate_b1, gate_W2, gate_b2,
                expert_W, expert_b, gen_W, gen_b):
    """Host-side layout prep (reshape/pad/cast/weight-fold). Returns in_maps."""
    f32 = np.float32
    bf16 = ml_dtypes.bfloat16

    # fused expert weights A_e = gen_W + expert_W[e]; ones-row bias.
    A = np.empty((E, K, D), dtype=f32)
    A[:, :CF, :] = np.asarray(expert_W, dtype=f32) + np.asarray(gen_W, dtype=f32)
    A[:, CF, :] = np.asarray(expert_b, dtype=f32) + np.asarray(gen_b, dtype=f32)
    Abf = A.astype(bf16)
    amain = np.ascontiguousarray(
        Abf[:, :896, :].reshape(E, 7, 128, D).transpose(2, 0, 1, 3))
    arem = np.ascontiguousarray(Abf[:, 896:K, :].transpose(1, 0, 2))

    # x transposed with ones-row, partition-major.
    x = np.asarray(cycle_curve_data, dtype=f32).reshape(B, L, CF)
    xT = np.empty((B, K, L), dtype=bf16)
    xT[:, :CF, :] = x.transpose(0, 2, 1).astype(bf16)
    xT[:, CF, :] = np.asarray(1.0, dtype=bf16)
    xmain = np.ascontiguousarray(
        xT[:, :896, :].reshape(B, 7, 128, L).transpose(0, 2, 1, 3))
    xrem = np.ascontiguousarray(xT[:, 896:K, :])

    # gating input, partition-major [128, 33*64].
    g = np.zeros((GK, B), dtype=f32)
    g[:DLLM, :] = np.asarray(DKP_embeddings, dtype=f32).T
    g[DLLM, :] = np.asarray(cycle_numbers, dtype=f32)[:, 0]
    g[DLLM + 1, :] = 1.0
    gintp = np.ascontiguousarray(
        g.reshape(GKT, 128, B).transpose(1, 0, 2).reshape(128, GKT * B)
        .astype(np.float16))

    W1p = np.zeros((GK, DFF), dtype=f32)
    W1p[:DLLM + 1, :] = np.asarray(gate_W1, dtype=f32)
    W1p[DLLM + 1, :] = np.asarray(gate_b1, dtype=f32)

    w2 = np.asarray(gate_W2, dtype=f32)
    b2rep = np.tile(np.asarray(gate_b2, dtype=f32).reshape(1, E), (BPC, 1))

    in_maps = []
    for c in range(NCORES):
        chunk = W1p[:, c * DFFC:(c + 1) * DFFC]
        w1pm = chunk.reshape(GKT, 128, DFFC).transpose(1, 0, 2)
        w1a = np.ascontiguousarray(w1pm[:, :GA, :].reshape(128, GA * DFFC).astype(np.float16))
        w1b = np.ascontiguousarray(w1pm[:, GA:, :].reshape(128, GB * DFFC).astype(np.float16))
        w2pm = np.ascontiguousarray(
            w2[c * DFFC:(c + 1) * DFFC, :].reshape(2, 128, E)
            .transpose(1, 0, 2).reshape(128, 2 * E))
        sel = np.zeros((B, BPC), dtype=f32)
        for i in range(BPC):
            sel[c * BPC + i, i] = 1.0
        sel2 = np.zeros((2, 256), dtype=f32)
        sel2[0, 0:128] = 1.0
        sel2[1, 128:256] = 1.0
        in_maps.append({
            "xmain": np.ascontiguousarray(xmain[c * BPC:(c + 1) * BPC]),
            "xrem": np.ascontiguousarray(xrem[c * BPC:(c + 1) * BPC]),
            "amain": amain,
            "arem": arem,
            "gintp": gintp,
            "w1a": w1a,
            "w1b": w1b,
            "w2p": w2pm,
            "selt": sel,
            "b2rep": b2rep,
            "sel2": sel2,
        })
    return in_maps


_CACHED = {}


def run(inputs, trace=False, tmpdir=None):
    """Run on the 8 NeuronCores; returns (full_output, BassKernelResults)."""
    from concourse import bass_utils
    in_maps = prep_inputs(**inputs)
    nc = _CACHED.get("nc")
    if nc is None:
        nc = make_nc()
        _CACHED["nc"] = nc
    res = bass_utils.run_bass_kernel_spmd(
        nc, in_maps, core_ids=list(range(NCORES)), trace=trace, tmpdir=tmpdir
    )
    outs = [np.asarray(r["out"]) for r in res.results]
    full = np.concatenate(outs, axis=0)          # [B, 128, MT, D] bf16
    full = full.transpose(0, 2, 1, 3).reshape(B, L, D).astype(np.float32)
    return full, res


def kernel(**inputs):
    full, _ = run(inputs, trace=False)
    return full
